# revision 18
# baseline (speedup 1.0000x reference)
"""L2-distance attention layer on 8 Trainium2 NeuronCores.

Sharding: data-parallel over batch B=8 (one sample per core); weights
replicated. The per-core Bass kernel computes t = wt @ (softmax(-l2) @ v)
and returns it; BatchNorm (global stats over B and N), the ReLU and the
residual add run on the host in fp32. Moving BN to the host removes the
on-device AllReduce, which makes the 8 cores fully independent — each
core's upload/compute/download pipelines through the device link
concurrently with the others.

Wire format (link is the bottleneck at ~40 MB/s):
  - x up: int8 with a per-(batch, point) column scale (scale = colmax/127),
    dequantized on device into fp16.
  - t down: uint8 with a per-(core, channel) affine (offset = mu - QR*sd,
    scale = 2*QR*sd / 254), computed on device from local channel stats.
    Any systematic half-step rounding bias is per-channel constant and
    cancels inside train-mode BatchNorm.
  - weights: fp16 (wq, wv) / fp32 (wt), uploaded once and cached on device.

Math notes (validated against the reference):
  - The L2 distance matrix is symmetric with exactly-zero diagonal, so
    softmax(-l2) needs no row-max subtraction (row max is always 0).
  - d2 is computed in ONE matmul per tile via augmented vectors:
    [q; sq; 1]^T [-2q; 1; sq] -> sq_j - 2 q_j.q_i + sq_i.
  - attention rows sum to 1, so bv shifts t by a per-channel constant;
    per-channel constants cancel inside train-mode BatchNorm, as does bt.
"""
import sys
sys.path.insert(0, '/opt/trn_rl_repo')
import numpy as np

B, C, N = 8, 256, 2048
C4 = C // 4          # 64
P = 128
JC = N // P          # 16 j-chunks
NB = N // 512        # 4 i-blocks
NCORES = 8
BN_EPS = 1e-5
QR = 4.0             # t quantization half-range in channel sigmas (MSE-optimal
                     # clip point for 8-bit uniform quantization of a Gaussian)

_CACHE = {}


def _build():
    import concourse.bass as bass
    import concourse.tile as tile
    from concourse import bacc, mybir
    f32 = mybir.dt.float32
    f16 = mybir.dt.float16
    u8 = mybir.dt.uint8

    nc = bacc.Bacc("TRN2", target_bir_lowering=False, debug=False,
                   num_devices=1)
    x_d = nc.dram_tensor("x", [P, 2, N], mybir.dt.int8, kind="ExternalInput")
    xs_d = nc.dram_tensor("xs", [N], f16, kind="ExternalInput")
    wq_d = nc.dram_tensor("wqT", [P, 2, C4], f16, kind="ExternalInput")
    wv_d = nc.dram_tensor("wvT", [P, 2, C], f16, kind="ExternalInput")
    wt_d = nc.dram_tensor("wtT", [P, 2, C], f32, kind="ExternalInput")
    eye_d = nc.dram_tensor("eyem", [P, P], u8, kind="ExternalInput")
    out_d = nc.dram_tensor("out", [P, 2, N], u8, kind="ExternalOutput")
    ts_d = nc.dram_tensor("tstat", [P, 4], f32, kind="ExternalOutput")

    AF = mybir.ActivationFunctionType
    OP = mybir.AluOpType

    with tile.TileContext(nc) as tc:
        with tc.tile_pool(name="perm", bufs=1) as perm, \
             tc.tile_pool(name="big", bufs=1) as bigp, \
             tc.tile_pool(name="dram", bufs=1, space="DRAM") as dram:
            # ---- permanent small tiles
            xw = perm.tile([P, 2, N], f16)
            wq = perm.tile([P, 2, C4], f16)
            nc.sync.dma_start(wq[:], wq_d.ap())
            wv = perm.tile([P, 2, C], f16)
            nc.sync.dma_start(wv[:], wv_d.ap())
            wt = perm.tile([P, 2, C], f32)
            nc.sync.dma_start(wt[:], wt_d.ap())
            eye = perm.tile([P, P], u8)
            nc.sync.dma_start(eye[:], eye_d.ap())
            zer = perm.tile([P, P], f32)
            nc.vector.memset(zer[:], 0.0)
            ones64 = perm.tile([C4, 1], f32)
            nc.vector.memset(ones64[:], 1.0)
            vT = perm.tile([P, JC, C], f32)
            dencol = perm.tile([P, JC], f32)
            rep = perm.tile([P, N], f32)
            l2big = bigp.tile([P, JC, N], f32)   # 8 KB/part * 16 = 128 KB/part
            xr = perm.tile([P, 2, N], f32)

            # ---- dequantize x: xw = f16(x8) * colscale
            _xpool = tc.tile_pool(name="xq", bufs=1)
            xqp = _xpool.__enter__()
            x8 = xqp.tile([P, 2, N], mybir.dt.int8)
            nc.sync.dma_start(x8[:], x_d.ap())
            xsr = xqp.tile([P, N], f16)
            xs_ap = xs_d.ap()
            bxs = bass.AP(tensor=xs_ap.tensor, offset=xs_ap.offset,
                          ap=[[0, P], [1, N]])
            nc.sync.dma_start(xsr[:], bxs)
            for oc in range(2):
                nc.vector.tensor_copy(out=xw[:, oc, :], in_=x8[:, oc, :])
                nc.vector.tensor_tensor(out=xw[:, oc, :], in0=xw[:, oc, :],
                                        in1=xsr[:], op=OP.mult)
            _xpool.__exit__(None, None, None)

            # ---- setup: q, sq, A/B bases, vT
            _ABpool = tc.tile_pool(name="ab", bufs=1)
            abp = _ABpool.__enter__()
            At = abp.tile([P, N], f32, tag="A", name="At")
            Bt = abp.tile([P, N], f32, tag="B", name="Bt")
            with tc.tile_pool(name="ps_set", bufs=2, space="PSUM") as pss:
                nc.vector.memset(At[:], 0.0)
                nc.vector.memset(Bt[:], 0.0)
                for nb in range(NB):
                    pq = pss.tile([C4, 512], f32, tag="pq")
                    nc.tensor.matmul(pq[:], lhsT=wq[:, 0, :],
                                     rhs=xw[:, 0, nb * 512:(nb + 1) * 512],
                                     start=True, stop=False)
                    nc.tensor.matmul(pq[:], lhsT=wq[:, 1, :],
                                     rhs=xw[:, 1, nb * 512:(nb + 1) * 512],
                                     start=False, stop=True)
                    nc.vector.tensor_copy(out=At[0:C4, nb * 512:(nb + 1) * 512],
                                          in_=pq[:])
                # q^2 into B rows 0:64 (scratch), then sq row
                nc.vector.tensor_tensor(out=Bt[0:C4, :], in0=At[0:C4, :],
                                        in1=At[0:C4, :], op=OP.mult)
                for nb in range(NB):
                    psq = pss.tile([1, 512], f32, tag="psq")
                    nc.tensor.matmul(psq[:],
                                     lhsT=ones64[:], rhs=Bt[0:C4, nb * 512:(nb + 1) * 512],
                                     start=True, stop=True)
                    nc.vector.tensor_copy(out=At[C4:C4 + 1, nb * 512:(nb + 1) * 512], in_=psq[:])
                    nc.vector.tensor_copy(out=Bt[96:97, nb * 512:(nb + 1) * 512], in_=psq[:])
                # overwrite B rows 0:64 with -2q (after sq matmuls read them)
                nc.vector.tensor_scalar(out=Bt[0:C4, :], in0=At[0:C4, :],
                                        scalar1=-2.0, scalar2=0.0,
                                        op0=OP.mult, op1=OP.add)
                nc.vector.memset(At[96:97, :], 1.0)
                nc.vector.memset(Bt[C4:C4 + 1, :], 1.0)
                # vT
                for jc in range(JC):
                    pv = pss.tile([P, C], f32, tag="pv")
                    nc.tensor.matmul(pv[:], lhsT=xw[:, 0, jc * P:(jc + 1) * P],
                                     rhs=wv[:, 0, :], start=True, stop=False)
                    nc.tensor.matmul(pv[:], lhsT=xw[:, 1, jc * P:(jc + 1) * P],
                                     rhs=wv[:, 1, :], start=False, stop=True)
                    nc.vector.tensor_copy(out=vT[:, jc, :], in_=pv[:])

            # ---- phase A: d2 tiles -> sqrt -> l2big
            with tc.tile_pool(name="ps_d2", bufs=2, space="PSUM") as psd:
                for a in range(JC):
                    pd2 = psd.tile([P, N], f32, tag="d2")
                    for nb in range(NB):
                        nc.tensor.matmul(pd2[:, nb * 512:(nb + 1) * 512],
                                         lhsT=At[:, a * P:(a + 1) * P],
                                         rhs=Bt[:, nb * 512:(nb + 1) * 512],
                                         start=True, stop=True)
                    nc.scalar.activation(l2big[:, a, :], pd2[:], AF.Sqrt)
                    # exact-zero the diagonal block (kills NaN from sqrt(neg))
                    nc.vector.copy_predicated(
                        out=l2big[:, a, a * P:(a + 1) * P],
                        mask=eye[:], data=zer[:])

            _ABpool.__exit__(None, None, None)
            # ---- phase B: exp (+den accum) and attn@v
            with tc.tile_pool(name="post", bufs=1) as postp:
                u8out = postp.tile([P, 2, N], u8)
                tstat = postp.tile([P, 4], f32)
                psav_cm = tc.tile_pool(name="ps_av", bufs=1, space="PSUM")
                psav = psav_cm.__enter__()
                pav = [psav.tile([P, 512], f32, tag=f"av{i}", name=f"pav{i}") for i in range(8)]
                for a in range(JC):
                    Pst = l2big[:, a, :]
                    nc.scalar.activation(Pst, l2big[:, a, :], AF.Exp,
                                         scale=-1.0,
                                         accum_out=dencol[:, a:a + 1])
                    for oc in range(2):
                        for ib in range(NB):
                            nc.tensor.matmul(
                                pav[oc * NB + ib][:],
                                lhsT=vT[:, a, oc * P:(oc + 1) * P],
                                rhs=Pst[:, ib * 512:(ib + 1) * 512],
                                start=(a == 0), stop=(a == JC - 1))

                # ---- denominators -> reciprocal -> broadcast row
                rden = perm.tile([P, JC], f32)
                nc.vector.reciprocal(rden[:], dencol[:])
                dden = dram.tile([N], f32)
                nc.sync.dma_start(dden.rearrange("(a r) -> r a", r=P), rden[:])
                bsrc = bass.AP(tensor=dden.tensor, offset=dden.offset,
                               ap=[[0, P], [1, N]])
                nc.sync.dma_start(rep[:], bsrc)

                # ---- x_r = pav * rep (normalize)
                for oc in range(2):
                    for ib in range(NB):
                        nc.vector.tensor_tensor(
                            out=xr[:, oc, ib * 512:(ib + 1) * 512],
                            in0=pav[oc * NB + ib][:],
                            in1=rep[:, ib * 512:(ib + 1) * 512], op=OP.mult)

                psav_cm.__exit__(None, None, None)
                # ---- t = wtT . xr (in place into xr, with s1 accumulation)
                s1p = [[postp.tile([P, 1], f32, name=f"s1_{o}_{n}", tag=f"s1_{o}_{n}")
                        for n in range(NB)] for o in range(2)]
                with tc.tile_pool(name="ps_t", bufs=2, space="PSUM") as pst:
                    for nb in range(NB):
                        ptl = []
                        for oc2 in range(2):
                            pt = pst.tile([P, 512], f32, tag=f"t{oc2}", name=f"pt{oc2}")
                            nc.tensor.matmul(pt[:], lhsT=wt[:, 0, oc2 * P:(oc2 + 1) * P],
                                             rhs=xr[:, 0, nb * 512:(nb + 1) * 512],
                                             start=True, stop=False)
                            nc.tensor.matmul(pt[:], lhsT=wt[:, 1, oc2 * P:(oc2 + 1) * P],
                                             rhs=xr[:, 1, nb * 512:(nb + 1) * 512],
                                             start=False, stop=True)
                            ptl.append(pt)
                        for oc2 in range(2):
                            nc.vector.tensor_scalar(
                                out=xr[:, oc2, nb * 512:(nb + 1) * 512],
                                in0=ptl[oc2][:], scalar1=1.0, scalar2=0.0,
                                op0=OP.mult, op1=OP.add,
                                accum_out=s1p[oc2][nb][:])

                # ---- per-channel stats: s1, s2 -> mu, sd -> offset/scale
                st = postp.tile([P, 8], f32)
                for oc2 in range(2):
                    nc.vector.tensor_tensor(out=st[:, oc2:oc2 + 1],
                                            in0=s1p[oc2][0][:], in1=s1p[oc2][1][:],
                                            op=OP.add)
                    nc.vector.tensor_tensor(out=st[:, oc2:oc2 + 1],
                                            in0=st[:, oc2:oc2 + 1], in1=s1p[oc2][2][:],
                                            op=OP.add)
                    nc.vector.tensor_tensor(out=st[:, oc2:oc2 + 1],
                                            in0=st[:, oc2:oc2 + 1], in1=s1p[oc2][3][:],
                                            op=OP.add)
                    # s2 via accumulating square pass (scratch into l2big)
                    nc.vector.scalar_tensor_tensor(
                        out=l2big[:, oc2, :], in0=xr[:, oc2, :], scalar=1.0,
                        in1=xr[:, oc2, :], op0=OP.mult, op1=OP.mult,
                        accum_out=st[:, 2 + oc2:3 + oc2])

                INV_N = 1.0 / N
                mu = postp.tile([P, 2], f32)
                sd = postp.tile([P, 2], f32)
                isc = postp.tile([P, 2], f32)
                qb = postp.tile([P, 2], f32)
                epst = postp.tile([P, 1], f32)
                nc.vector.memset(epst[:], 1e-12)
                for oc2 in range(2):
                    nc.vector.tensor_scalar(out=mu[:, oc2:oc2 + 1],
                                            in0=st[:, oc2:oc2 + 1],
                                            scalar1=INV_N, scalar2=0.0,
                                            op0=OP.mult, op1=OP.add)
                    # var = s2/N - mu^2
                    nc.vector.scalar_tensor_tensor(
                        out=sd[:, oc2:oc2 + 1], in0=mu[:, oc2:oc2 + 1],
                        scalar=-1.0, in1=mu[:, oc2:oc2 + 1],
                        op0=OP.mult, op1=OP.mult)
                    nc.vector.scalar_tensor_tensor(
                        out=sd[:, oc2:oc2 + 1], in0=st[:, 2 + oc2:3 + oc2],
                        scalar=INV_N, in1=sd[:, oc2:oc2 + 1],
                        op0=OP.mult, op1=OP.subtract)
                    # sd = sqrt(var) (+tiny eps to avoid 0)
                    nc.scalar.activation(sd[:, oc2:oc2 + 1], sd[:, oc2:oc2 + 1],
                                         AF.Sqrt, bias=epst[:])
                    # isc = 254/(2*QR*sd); offset = mu - QR*sd
                    nc.vector.tensor_scalar(out=isc[:, oc2:oc2 + 1],
                                            in0=sd[:, oc2:oc2 + 1],
                                            scalar1=(2.0 * QR) / 254.0, scalar2=0.0,
                                            op0=OP.mult, op1=OP.add)
                    nc.vector.reciprocal(isc[:, oc2:oc2 + 1], isc[:, oc2:oc2 + 1])
                    # tstat columns: [off0, sc0, off1, sc1]
                    nc.vector.scalar_tensor_tensor(
                        out=tstat[:, 2 * oc2:2 * oc2 + 1], in0=sd[:, oc2:oc2 + 1],
                        scalar=-QR, in1=mu[:, oc2:oc2 + 1],
                        op0=OP.mult, op1=OP.add)
                    nc.vector.tensor_scalar(out=tstat[:, 2 * oc2 + 1:2 * oc2 + 2],
                                            in0=sd[:, oc2:oc2 + 1],
                                            scalar1=(2.0 * QR) / 254.0, scalar2=0.0,
                                            op0=OP.mult, op1=OP.add)
                    # qb = -off*isc + 0.5  (so u = t*isc + qb)
                    nc.vector.tensor_tensor(out=qb[:, oc2:oc2 + 1],
                                            in0=tstat[:, 2 * oc2:2 * oc2 + 1],
                                            in1=isc[:, oc2:oc2 + 1], op=OP.mult)
                    nc.vector.tensor_scalar(out=qb[:, oc2:oc2 + 1],
                                            in0=qb[:, oc2:oc2 + 1],
                                            scalar1=-1.0, scalar2=0.5,
                                            op0=OP.mult, op1=OP.add)

                # ---- quantize: u8 = min(relu(t*isc + qb), 254.99) truncated
                for oc2 in range(2):
                    z = l2big[:, 4 + oc2, :]
                    nc.scalar.activation(z, xr[:, oc2, :], AF.Relu,
                                         scale=isc[:, oc2:oc2 + 1],
                                         bias=qb[:, oc2:oc2 + 1])
                    nc.vector.tensor_scalar(out=u8out[:, oc2, :], in0=z,
                                            scalar1=254.99, scalar2=0.0,
                                            op0=OP.min, op1=OP.add)
                nc.sync.dma_start(out_d.ap(), u8out[:])
                nc.sync.dma_start(ts_d.ap(), tstat[:])

    nc.compile()
    return nc


def _get_nc():
    if "nc" not in _CACHE:
        _CACHE["nc"] = _build()
    return _CACHE["nc"]


def _host_weights():
    # per-core weight arrays (fp16 wire for wq/wv, fp32 for wt)
    if "weights" in _CACHE:
        return _CACHE["weights"]
    wq, wv, wt = _CACHE["_raw_w"]
    wqT = np.ascontiguousarray(
        np.asarray(wq, np.float32).T.reshape(2, P, C4).transpose(1, 0, 2)).astype(np.float16)
    wvT = np.ascontiguousarray(
        np.asarray(wv, np.float32).T.reshape(2, P, C).transpose(1, 0, 2)).astype(np.float16)
    wtT = np.ascontiguousarray(
        np.asarray(wt, np.float32).T.reshape(2, P, C).transpose(1, 0, 2))
    eyem = np.eye(P, dtype=np.uint8)
    _CACHE["weights"] = {"wqT": wqT, "wvT": wvT, "wtT": wtT, "eyem": eyem}
    return _CACHE["weights"]


def _quant_x_core(xb):
    """xb: [C, N] f32 for one batch -> (x8 [P,2,N] int8, xs [N] f16)."""
    colmax = np.abs(xb).max(axis=0)
    np.maximum(colmax, 1e-12, out=colmax)
    xs = (colmax / 127.0).astype(np.float16)
    inv = 127.0 / colmax
    xq = np.rint(xb.reshape(2, P, N).transpose(1, 0, 2) * inv)
    np.clip(xq, -127, 127, out=xq)
    return xq.astype(np.int8), xs


def _io_names(nc):
    from concourse import mybir
    import jax
    in_names, out_names, out_avals = [], [], []
    for alloc in nc.m.functions[0].allocations:
        if not isinstance(alloc, mybir.MemoryLocationSet):
            continue
        name = alloc.memorylocations[0].name
        if alloc.kind == "ExternalInput":
            in_names.append(name)
        elif alloc.kind == "ExternalOutput":
            out_names.append(name)
            out_avals.append(jax.core.ShapedArray(
                tuple(alloc.tensor_shape), mybir.dt.np(alloc.dtype)))
    return in_names, out_names, out_avals


def _get_runner_percore():
    """Per-device AOT executables: upload/compute/download pipeline per core."""
    if "runner_pc" in _CACHE:
        return _CACHE["runner_pc"]
    import jax
    from concurrent.futures import ThreadPoolExecutor
    from concourse import bass2jax
    bass2jax.install_neuronx_cc_hook()

    nc = _get_nc()
    in_names, out_names, out_avals = _io_names(nc)
    partition_name = nc.partition_id_tensor.name if nc.partition_id_tensor else None
    if partition_name is not None and partition_name in in_names:
        in_names.remove(partition_name)
    all_in_names = list(in_names)
    if partition_name is not None:
        all_in_names.append(partition_name)

    devs = jax.devices()[:NCORES]

    def _body(*args):
        operands = list(args)
        if partition_name is not None:
            operands.append(bass2jax.partition_id_tensor())
        outs = bass2jax._bass_exec_p.bind(
            *operands,
            out_avals=tuple(out_avals),
            in_names=tuple(all_in_names),
            out_names=tuple(out_names),
            lowering_input_output_aliases=(),
            sim_require_finite=True,
            sim_require_nnan=True,
            nc=nc,
        )
        return tuple(outs)

    wh = _host_weights()
    in_specs_np = {"x": np.zeros((P, 2, N), np.int8),
                   "xs": np.zeros((N,), np.float16)}

    def _compile_core(c):
        specs = []
        for name in in_names:
            arr = in_specs_np.get(name, wh.get(name))
            specs.append(jax.ShapeDtypeStruct(
                arr.shape, arr.dtype,
                sharding=jax.sharding.SingleDeviceSharding(devs[c])))
        try:
            comp = bass2jax.fast_dispatch_compile(
                lambda: jax.jit(_body, keep_unused=True).lower(*specs).compile())
        except Exception:
            comp = jax.jit(_body, keep_unused=True).lower(*specs).compile()
        wd = {name: jax.device_put(wh[name], devs[c])
              for name in in_names if name not in ("x", "xs")}
        return comp, wd

    cpool = ThreadPoolExecutor(NCORES)
    futs = [cpool.submit(_compile_core, c) for c in range(NCORES)]
    results = [f.result() for f in futs]
    compiled = [r0 for r0, _ in results]
    wdev = [r1 for _, r1 in results]

    runner = {
        "compiled": compiled, "devs": devs, "wdev": wdev,
        "in_names": in_names, "out_names": out_names,
        "pool": ThreadPoolExecutor(NCORES),
    }
    _CACHE["runner_pc"] = runner
    return runner


def _run_percore(x):
    """x: [B, C, N] f32. Returns per-core (t_f32 [P,2,N], s1 [P,2], s2 [P,2])."""
    import jax
    r = _get_runner_percore()
    devs, pool = r["devs"], r["pool"]
    oidx = {name: i for i, name in enumerate(r["out_names"])}

    def work(c):
        x8, xs = _quant_x_core(x[c])
        x_c = jax.device_put(x8, devs[c])
        xs_c = jax.device_put(xs, devs[c])
        args = []
        for name in r["in_names"]:
            if name == "x":
                args.append(x_c)
            elif name == "xs":
                args.append(xs_c)
            else:
                args.append(r["wdev"][c][name])
        outs = r["compiled"][c](*args)
        for o in outs:
            try:
                o.copy_to_host_async()
            except Exception:
                pass
        u8t = np.asarray(outs[oidx["out"]])
        tstat = np.asarray(outs[oidx["tstat"]])
        # dequantize + local BN partial sums here so the work overlaps the
        # other cores' still-running downloads
        t = u8t.astype(np.float32)               # [P, 2, N]
        t *= tstat[:, 1::2][..., None]
        t += tstat[:, 0::2][..., None]
        s1 = t.sum(axis=2)                       # [P, 2]
        s2 = np.einsum('pon,pon->po', t, t)      # [P, 2]
        return t, s1, s2

    futs = [pool.submit(work, c) for c in range(NCORES)]
    return [f.result() for f in futs]


def _run_spmd_fallback(x):
    from concourse.bass_utils import run_bass_kernel_spmd
    nc = _get_nc()
    wh = _host_weights()
    in_maps = []
    for c in range(NCORES):
        x8, xs = _quant_x_core(x[c])
        in_maps.append({"x": x8, "xs": xs, **wh})
    _CACHE["last_in_maps"] = in_maps
    res = run_bass_kernel_spmd(nc, in_maps, core_ids=list(range(NCORES)))
    _CACHE["last_res"] = res
    return (np.stack([res.results[c]["out"] for c in range(NCORES)]),
            np.stack([res.results[c]["tstat"] for c in range(NCORES)]))


def kernel(x, wq, wv, bv, wt, bt, gamma, beta):
    import hashlib
    x = np.asarray(x, dtype=np.float32)
    wfp = hashlib.md5(
        np.asarray(wq, np.float32).tobytes()
        + np.asarray(wv, np.float32).tobytes()
        + np.asarray(wt, np.float32).tobytes()).hexdigest()
    if _CACHE.get("wfp") != wfp:
        # weights changed (or first call): drop host + device weight caches
        _CACHE.pop("weights", None)
        _CACHE["_raw_w"] = (wq, wv, wt)
        _CACHE["wfp"] = wfp
        rpc = _CACHE.get("runner_pc")
        if rpc is not None:
            import jax
            wh = _host_weights()
            for c in range(NCORES):
                rpc["wdev"][c] = {
                    name: jax.device_put(wh[name], rpc["devs"][c])
                    for name in rpc["in_names"] if name not in ("x", "xs")}

    parts = None
    if _CACHE.get("_pc_fail_count", 0) < 2:
        try:
            parts = _run_percore(x)
            _CACHE["_pc_fail_count"] = 0
        except Exception as e:
            import traceback
            _CACHE["_pc_fail_count"] = _CACHE.get("_pc_fail_count", 0) + 1
            print("percore path failed, falling back:", repr(e)[:300],
                  file=sys.stderr)
            traceback.print_exc()
    if parts is None:
        u8t, tstat = _run_spmd_fallback(x)
        parts = []
        for b in range(B):
            t = u8t[b].astype(np.float32)
            t *= tstat[b][:, 1::2][..., None]
            t += tstat[b][:, 0::2][..., None]
            parts.append((t, t.sum(axis=2), np.einsum('pon,pon->po', t, t)))

    # ---- host tail: BatchNorm (global stats) + ReLU + residual
    from concurrent.futures import ThreadPoolExecutor
    pool = _CACHE.setdefault("_tail_pool", ThreadPoolExecutor(NCORES))
    tb = [p[0] for p in parts]
    M = B * N
    s1 = sum(p[1] for p in parts)
    s2 = sum(p[2] for p in parts)
    mean = s1 / M
    var = s2 / M - mean * mean
    rstd = 1.0 / np.sqrt(var + BN_EPS)
    g2 = np.asarray(gamma, np.float32).reshape(2, P).T   # [P, 2]
    b2 = np.asarray(beta, np.float32).reshape(2, P).T
    A = (g2 * rstd)[:, :, None]
    Bc = (b2 - mean * g2 * rstd)[:, :, None]
    out = np.empty((B, C, N), np.float32)

    def _apply(b):
        t = tb[b]
        t *= A
        t += Bc
        np.maximum(t, 0.0, out=t)
        ob = out[b].reshape(2, P, N)
        np.copyto(ob, t.transpose(1, 0, 2))
        ob += x[b].reshape(2, P, N)

    list(pool.map(_apply, range(B)))
    return out


# revision 20
# speedup vs baseline: 1.0295x; 1.0295x over previous
"""L2-distance attention layer on 8 Trainium2 NeuronCores.

Sharding: data-parallel over batch B=8 (one sample per core); weights
replicated. The per-core Bass kernel computes t = wt @ (softmax(-l2) @ v)
and returns it; BatchNorm (global stats over B and N), the ReLU and the
residual add run on the host in fp32. Moving BN to the host removes the
on-device AllReduce, which makes the 8 cores fully independent — each
core's upload/compute/download pipelines through the device link
concurrently with the others.

Wire format (link is the bottleneck at ~40 MB/s):
  - x up: int8 with a per-(batch, point) column scale (scale = colmax/127),
    dequantized on device into fp16.
  - t down: uint8 with a per-(core, channel) affine (offset = mu - QR*sd,
    scale = 2*QR*sd / 254), computed on device from local channel stats.
    Any systematic half-step rounding bias is per-channel constant and
    cancels inside train-mode BatchNorm.
  - weights: fp16 (wq, wv) / fp32 (wt), uploaded once and cached on device.

Math notes (validated against the reference):
  - The L2 distance matrix is symmetric with exactly-zero diagonal, so
    softmax(-l2) needs no row-max subtraction (row max is always 0).
  - d2 is computed in ONE matmul per tile via augmented vectors:
    [q; sq; 1]^T [-2q; 1; sq] -> sq_j - 2 q_j.q_i + sq_i.
  - attention rows sum to 1, so bv shifts t by a per-channel constant;
    per-channel constants cancel inside train-mode BatchNorm, as does bt.
"""
import sys
sys.path.insert(0, '/opt/trn_rl_repo')
import numpy as np

B, C, N = 8, 256, 2048
C4 = C // 4          # 64
P = 128
JC = N // P          # 16 j-chunks
NB = N // 512        # 4 i-blocks
NCORES = 8
BN_EPS = 1e-5
QR = 4.0             # t quantization half-range in channel sigmas (MSE-optimal
                     # clip point for 8-bit uniform quantization of a Gaussian)

_CACHE = {}


def _build():
    import concourse.bass as bass
    import concourse.tile as tile
    from concourse import bacc, mybir
    f32 = mybir.dt.float32
    f16 = mybir.dt.float16
    u8 = mybir.dt.uint8

    nc = bacc.Bacc("TRN2", target_bir_lowering=False, debug=False,
                   num_devices=1)
    x_d = nc.dram_tensor("x", [P, 2, N], mybir.dt.int8, kind="ExternalInput")
    xs_d = nc.dram_tensor("xs", [N], f16, kind="ExternalInput")
    wq_d = nc.dram_tensor("wqT", [P, 2, C4], f16, kind="ExternalInput")
    wv_d = nc.dram_tensor("wvT", [P, 2, C], f16, kind="ExternalInput")
    wt_d = nc.dram_tensor("wtT", [P, 2, C], f32, kind="ExternalInput")
    eye_d = nc.dram_tensor("eyem", [P, P], u8, kind="ExternalInput")
    out_d = nc.dram_tensor("out", [P, 2, N], u8, kind="ExternalOutput")
    ts_d = nc.dram_tensor("tstat", [P, 4], f32, kind="ExternalOutput")

    AF = mybir.ActivationFunctionType
    OP = mybir.AluOpType

    with tile.TileContext(nc) as tc:
        with tc.tile_pool(name="perm", bufs=1) as perm, \
             tc.tile_pool(name="big", bufs=1) as bigp, \
             tc.tile_pool(name="dram", bufs=1, space="DRAM") as dram:
            # ---- permanent small tiles
            xw = perm.tile([P, 2, N], f16)
            wq = perm.tile([P, 2, C4], f16)
            nc.sync.dma_start(wq[:], wq_d.ap())
            wv = perm.tile([P, 2, C], f16)
            nc.sync.dma_start(wv[:], wv_d.ap())
            wt = perm.tile([P, 2, C], f32)
            nc.sync.dma_start(wt[:], wt_d.ap())
            eye = perm.tile([P, P], u8)
            nc.sync.dma_start(eye[:], eye_d.ap())
            zer = perm.tile([P, P], f32)
            nc.vector.memset(zer[:], 0.0)
            ones64 = perm.tile([C4, 1], f32)
            nc.vector.memset(ones64[:], 1.0)
            vT = perm.tile([P, JC, C], f32)
            dencol = perm.tile([P, JC], f32)
            rep = perm.tile([P, N], f32)
            l2big = bigp.tile([P, JC, N], f32)   # 8 KB/part * 16 = 128 KB/part
            xr = perm.tile([P, 2, N], f32)

            # ---- dequantize x: xw = f16(x8) * colscale
            _xpool = tc.tile_pool(name="xq", bufs=1)
            xqp = _xpool.__enter__()
            x8 = xqp.tile([P, 2, N], mybir.dt.int8)
            nc.sync.dma_start(x8[:], x_d.ap())
            xsr = xqp.tile([P, N], f16)
            xs_ap = xs_d.ap()
            bxs = bass.AP(tensor=xs_ap.tensor, offset=xs_ap.offset,
                          ap=[[0, P], [1, N]])
            nc.sync.dma_start(xsr[:], bxs)
            for oc in range(2):
                nc.vector.tensor_copy(out=xw[:, oc, :], in_=x8[:, oc, :])
                nc.vector.tensor_tensor(out=xw[:, oc, :], in0=xw[:, oc, :],
                                        in1=xsr[:], op=OP.mult)
            _xpool.__exit__(None, None, None)

            # ---- setup: q, sq, A/B bases, vT
            _ABpool = tc.tile_pool(name="ab", bufs=1)
            abp = _ABpool.__enter__()
            At = abp.tile([P, N], f32, tag="A", name="At")
            Bt = abp.tile([P, N], f32, tag="B", name="Bt")
            with tc.tile_pool(name="ps_set", bufs=2, space="PSUM") as pss:
                nc.vector.memset(At[:], 0.0)
                nc.vector.memset(Bt[:], 0.0)
                for nb in range(NB):
                    pq = pss.tile([C4, 512], f32, tag="pq")
                    nc.tensor.matmul(pq[:], lhsT=wq[:, 0, :],
                                     rhs=xw[:, 0, nb * 512:(nb + 1) * 512],
                                     start=True, stop=False)
                    nc.tensor.matmul(pq[:], lhsT=wq[:, 1, :],
                                     rhs=xw[:, 1, nb * 512:(nb + 1) * 512],
                                     start=False, stop=True)
                    nc.vector.tensor_copy(out=At[0:C4, nb * 512:(nb + 1) * 512],
                                          in_=pq[:])
                # q^2 into B rows 0:64 (scratch), then sq row
                nc.vector.tensor_tensor(out=Bt[0:C4, :], in0=At[0:C4, :],
                                        in1=At[0:C4, :], op=OP.mult)
                for nb in range(NB):
                    psq = pss.tile([1, 512], f32, tag="psq")
                    nc.tensor.matmul(psq[:],
                                     lhsT=ones64[:], rhs=Bt[0:C4, nb * 512:(nb + 1) * 512],
                                     start=True, stop=True)
                    nc.vector.tensor_copy(out=At[C4:C4 + 1, nb * 512:(nb + 1) * 512], in_=psq[:])
                    nc.vector.tensor_copy(out=Bt[96:97, nb * 512:(nb + 1) * 512], in_=psq[:])
                # overwrite B rows 0:64 with -2q (after sq matmuls read them)
                nc.vector.tensor_scalar(out=Bt[0:C4, :], in0=At[0:C4, :],
                                        scalar1=-2.0, scalar2=0.0,
                                        op0=OP.mult, op1=OP.add)
                nc.vector.memset(At[96:97, :], 1.0)
                nc.vector.memset(Bt[C4:C4 + 1, :], 1.0)
                # vT
                for jc in range(JC):
                    pv = pss.tile([P, C], f32, tag="pv")
                    nc.tensor.matmul(pv[:], lhsT=xw[:, 0, jc * P:(jc + 1) * P],
                                     rhs=wv[:, 0, :], start=True, stop=False)
                    nc.tensor.matmul(pv[:], lhsT=xw[:, 1, jc * P:(jc + 1) * P],
                                     rhs=wv[:, 1, :], start=False, stop=True)
                    nc.vector.tensor_copy(out=vT[:, jc, :], in_=pv[:])

            # ---- phase A: d2 tiles -> sqrt -> l2big
            with tc.tile_pool(name="ps_d2", bufs=2, space="PSUM") as psd:
                for a in range(JC):
                    pd2 = psd.tile([P, N], f32, tag="d2")
                    for nb in range(NB):
                        nc.tensor.matmul(pd2[:, nb * 512:(nb + 1) * 512],
                                         lhsT=At[:, a * P:(a + 1) * P],
                                         rhs=Bt[:, nb * 512:(nb + 1) * 512],
                                         start=True, stop=True)
                    nc.scalar.activation(l2big[:, a, :], pd2[:], AF.Sqrt)
                    # exact-zero the diagonal block (kills NaN from sqrt(neg))
                    nc.vector.copy_predicated(
                        out=l2big[:, a, a * P:(a + 1) * P],
                        mask=eye[:], data=zer[:])

            _ABpool.__exit__(None, None, None)
            # ---- phase B: exp (+den accum) and attn@v
            with tc.tile_pool(name="post", bufs=1) as postp:
                u8out = postp.tile([P, 2, N], u8)
                tstat = postp.tile([P, 4], f32)
                psav_cm = tc.tile_pool(name="ps_av", bufs=1, space="PSUM")
                psav = psav_cm.__enter__()
                pav = [psav.tile([P, 512], f32, tag=f"av{i}", name=f"pav{i}") for i in range(8)]
                for a in range(JC):
                    Pst = l2big[:, a, :]
                    nc.scalar.activation(Pst, l2big[:, a, :], AF.Exp,
                                         scale=-1.0,
                                         accum_out=dencol[:, a:a + 1])
                    for oc in range(2):
                        for ib in range(NB):
                            nc.tensor.matmul(
                                pav[oc * NB + ib][:],
                                lhsT=vT[:, a, oc * P:(oc + 1) * P],
                                rhs=Pst[:, ib * 512:(ib + 1) * 512],
                                start=(a == 0), stop=(a == JC - 1))

                # ---- denominators -> reciprocal -> broadcast row
                rden = perm.tile([P, JC], f32)
                nc.vector.reciprocal(rden[:], dencol[:])
                dden = dram.tile([N], f32)
                nc.sync.dma_start(dden.rearrange("(a r) -> r a", r=P), rden[:])
                bsrc = bass.AP(tensor=dden.tensor, offset=dden.offset,
                               ap=[[0, P], [1, N]])
                nc.sync.dma_start(rep[:], bsrc)

                # ---- x_r = pav * rep (normalize)
                for oc in range(2):
                    for ib in range(NB):
                        nc.vector.tensor_tensor(
                            out=xr[:, oc, ib * 512:(ib + 1) * 512],
                            in0=pav[oc * NB + ib][:],
                            in1=rep[:, ib * 512:(ib + 1) * 512], op=OP.mult)

                psav_cm.__exit__(None, None, None)
                # ---- t = wtT . xr (in place into xr, with s1 accumulation)
                s1p = [[postp.tile([P, 1], f32, name=f"s1_{o}_{n}", tag=f"s1_{o}_{n}")
                        for n in range(NB)] for o in range(2)]
                with tc.tile_pool(name="ps_t", bufs=2, space="PSUM") as pst:
                    for nb in range(NB):
                        ptl = []
                        for oc2 in range(2):
                            pt = pst.tile([P, 512], f32, tag=f"t{oc2}", name=f"pt{oc2}")
                            nc.tensor.matmul(pt[:], lhsT=wt[:, 0, oc2 * P:(oc2 + 1) * P],
                                             rhs=xr[:, 0, nb * 512:(nb + 1) * 512],
                                             start=True, stop=False)
                            nc.tensor.matmul(pt[:], lhsT=wt[:, 1, oc2 * P:(oc2 + 1) * P],
                                             rhs=xr[:, 1, nb * 512:(nb + 1) * 512],
                                             start=False, stop=True)
                            ptl.append(pt)
                        for oc2 in range(2):
                            nc.vector.tensor_scalar(
                                out=xr[:, oc2, nb * 512:(nb + 1) * 512],
                                in0=ptl[oc2][:], scalar1=1.0, scalar2=0.0,
                                op0=OP.mult, op1=OP.add,
                                accum_out=s1p[oc2][nb][:])

                # ---- per-channel stats: s1, s2 -> mu, sd -> offset/scale
                st = postp.tile([P, 8], f32)
                for oc2 in range(2):
                    nc.vector.tensor_tensor(out=st[:, oc2:oc2 + 1],
                                            in0=s1p[oc2][0][:], in1=s1p[oc2][1][:],
                                            op=OP.add)
                    nc.vector.tensor_tensor(out=st[:, oc2:oc2 + 1],
                                            in0=st[:, oc2:oc2 + 1], in1=s1p[oc2][2][:],
                                            op=OP.add)
                    nc.vector.tensor_tensor(out=st[:, oc2:oc2 + 1],
                                            in0=st[:, oc2:oc2 + 1], in1=s1p[oc2][3][:],
                                            op=OP.add)
                    # s2 via accumulating square pass (scratch into l2big)
                    nc.vector.scalar_tensor_tensor(
                        out=l2big[:, oc2, :], in0=xr[:, oc2, :], scalar=1.0,
                        in1=xr[:, oc2, :], op0=OP.mult, op1=OP.mult,
                        accum_out=st[:, 2 + oc2:3 + oc2])

                INV_N = 1.0 / N
                mu = postp.tile([P, 2], f32)
                sd = postp.tile([P, 2], f32)
                isc = postp.tile([P, 2], f32)
                qb = postp.tile([P, 2], f32)
                epst = postp.tile([P, 1], f32)
                nc.vector.memset(epst[:], 1e-12)
                for oc2 in range(2):
                    nc.vector.tensor_scalar(out=mu[:, oc2:oc2 + 1],
                                            in0=st[:, oc2:oc2 + 1],
                                            scalar1=INV_N, scalar2=0.0,
                                            op0=OP.mult, op1=OP.add)
                    # var = s2/N - mu^2
                    nc.vector.scalar_tensor_tensor(
                        out=sd[:, oc2:oc2 + 1], in0=mu[:, oc2:oc2 + 1],
                        scalar=-1.0, in1=mu[:, oc2:oc2 + 1],
                        op0=OP.mult, op1=OP.mult)
                    nc.vector.scalar_tensor_tensor(
                        out=sd[:, oc2:oc2 + 1], in0=st[:, 2 + oc2:3 + oc2],
                        scalar=INV_N, in1=sd[:, oc2:oc2 + 1],
                        op0=OP.mult, op1=OP.subtract)
                    # sd = sqrt(var) (+tiny eps to avoid 0)
                    nc.scalar.activation(sd[:, oc2:oc2 + 1], sd[:, oc2:oc2 + 1],
                                         AF.Sqrt, bias=epst[:])
                    # isc = 254/(2*QR*sd); offset = mu - QR*sd
                    nc.vector.tensor_scalar(out=isc[:, oc2:oc2 + 1],
                                            in0=sd[:, oc2:oc2 + 1],
                                            scalar1=(2.0 * QR) / 254.0, scalar2=0.0,
                                            op0=OP.mult, op1=OP.add)
                    nc.vector.reciprocal(isc[:, oc2:oc2 + 1], isc[:, oc2:oc2 + 1])
                    # tstat columns: [off0, sc0, off1, sc1]
                    nc.vector.scalar_tensor_tensor(
                        out=tstat[:, 2 * oc2:2 * oc2 + 1], in0=sd[:, oc2:oc2 + 1],
                        scalar=-QR, in1=mu[:, oc2:oc2 + 1],
                        op0=OP.mult, op1=OP.add)
                    nc.vector.tensor_scalar(out=tstat[:, 2 * oc2 + 1:2 * oc2 + 2],
                                            in0=sd[:, oc2:oc2 + 1],
                                            scalar1=(2.0 * QR) / 254.0, scalar2=0.0,
                                            op0=OP.mult, op1=OP.add)
                    # qb = -off*isc + 0.5  (so u = t*isc + qb)
                    nc.vector.tensor_tensor(out=qb[:, oc2:oc2 + 1],
                                            in0=tstat[:, 2 * oc2:2 * oc2 + 1],
                                            in1=isc[:, oc2:oc2 + 1], op=OP.mult)
                    nc.vector.tensor_scalar(out=qb[:, oc2:oc2 + 1],
                                            in0=qb[:, oc2:oc2 + 1],
                                            scalar1=-1.0, scalar2=0.5,
                                            op0=OP.mult, op1=OP.add)

                # ---- quantize: u8 = min(relu(t*isc + qb), 254.99) truncated
                for oc2 in range(2):
                    z = l2big[:, 4 + oc2, :]
                    nc.scalar.activation(z, xr[:, oc2, :], AF.Relu,
                                         scale=isc[:, oc2:oc2 + 1],
                                         bias=qb[:, oc2:oc2 + 1])
                    nc.vector.tensor_scalar(out=u8out[:, oc2, :], in0=z,
                                            scalar1=254.99, scalar2=0.0,
                                            op0=OP.min, op1=OP.add)
                nc.sync.dma_start(out_d.ap(), u8out[:])
                nc.sync.dma_start(ts_d.ap(), tstat[:])

    nc.compile()
    return nc


def _get_nc():
    if "nc" not in _CACHE:
        _CACHE["nc"] = _build()
    return _CACHE["nc"]


def _host_weights():
    # per-core weight arrays (fp16 wire for wq/wv, fp32 for wt)
    if "weights" in _CACHE:
        return _CACHE["weights"]
    wq, wv, wt = _CACHE["_raw_w"]
    wqT = np.ascontiguousarray(
        np.asarray(wq, np.float32).T.reshape(2, P, C4).transpose(1, 0, 2)).astype(np.float16)
    wvT = np.ascontiguousarray(
        np.asarray(wv, np.float32).T.reshape(2, P, C).transpose(1, 0, 2)).astype(np.float16)
    wtT = np.ascontiguousarray(
        np.asarray(wt, np.float32).T.reshape(2, P, C).transpose(1, 0, 2))
    eyem = np.eye(P, dtype=np.uint8)
    _CACHE["weights"] = {"wqT": wqT, "wvT": wvT, "wtT": wtT, "eyem": eyem}
    return _CACHE["weights"]


def _quant_x_core(xb):
    """xb: [C, N] f32 for one batch -> (x8 [P,2,N] int8, xs [N] f16)."""
    colmax = np.abs(xb).max(axis=0)
    np.maximum(colmax, 1e-12, out=colmax)
    xs = (colmax / 127.0).astype(np.float16)
    inv = 127.0 / colmax
    xq = np.rint(xb.reshape(2, P, N).transpose(1, 0, 2) * inv)
    np.clip(xq, -127, 127, out=xq)
    return xq.astype(np.int8), xs


def _io_names(nc):
    from concourse import mybir
    import jax
    in_names, out_names, out_avals = [], [], []
    for alloc in nc.m.functions[0].allocations:
        if not isinstance(alloc, mybir.MemoryLocationSet):
            continue
        name = alloc.memorylocations[0].name
        if alloc.kind == "ExternalInput":
            in_names.append(name)
        elif alloc.kind == "ExternalOutput":
            out_names.append(name)
            out_avals.append(jax.core.ShapedArray(
                tuple(alloc.tensor_shape), mybir.dt.np(alloc.dtype)))
    return in_names, out_names, out_avals


def _get_runner_percore():
    """Per-device AOT executables: upload/compute/download pipeline per core."""
    if "runner_pc" in _CACHE:
        return _CACHE["runner_pc"]
    import jax
    from concurrent.futures import ThreadPoolExecutor
    from concourse import bass2jax
    bass2jax.install_neuronx_cc_hook()

    nc = _get_nc()
    in_names, out_names, out_avals = _io_names(nc)
    partition_name = nc.partition_id_tensor.name if nc.partition_id_tensor else None
    if partition_name is not None and partition_name in in_names:
        in_names.remove(partition_name)
    all_in_names = list(in_names)
    if partition_name is not None:
        all_in_names.append(partition_name)

    devs = jax.devices()[:NCORES]

    def _body(*args):
        operands = list(args)
        if partition_name is not None:
            operands.append(bass2jax.partition_id_tensor())
        outs = bass2jax._bass_exec_p.bind(
            *operands,
            out_avals=tuple(out_avals),
            in_names=tuple(all_in_names),
            out_names=tuple(out_names),
            lowering_input_output_aliases=(),
            sim_require_finite=True,
            sim_require_nnan=True,
            nc=nc,
        )
        return tuple(outs)

    wh = _host_weights()
    in_specs_np = {"x": np.zeros((P, 2, N), np.int8),
                   "xs": np.zeros((N,), np.float16)}

    def _compile_core(c):
        specs = []
        for name in in_names:
            arr = in_specs_np.get(name, wh.get(name))
            specs.append(jax.ShapeDtypeStruct(
                arr.shape, arr.dtype,
                sharding=jax.sharding.SingleDeviceSharding(devs[c])))
        try:
            comp = bass2jax.fast_dispatch_compile(
                lambda: jax.jit(_body, keep_unused=True).lower(*specs).compile())
        except Exception:
            comp = jax.jit(_body, keep_unused=True).lower(*specs).compile()
        wd = {name: jax.device_put(wh[name], devs[c])
              for name in in_names if name not in ("x", "xs")}
        return comp, wd

    cpool = ThreadPoolExecutor(NCORES)
    futs = [cpool.submit(_compile_core, c) for c in range(NCORES)]
    results = [f.result() for f in futs]
    compiled = [r0 for r0, _ in results]
    wdev = [r1 for _, r1 in results]

    runner = {
        "compiled": compiled, "devs": devs, "wdev": wdev,
        "in_names": in_names, "out_names": out_names,
        "pool": ThreadPoolExecutor(NCORES),
    }
    _CACHE["runner_pc"] = runner
    return runner


def _run_percore(x):
    """x: [B, C, N] f32. Returns per-core (t_f32 [P,2,N], s1 [P,2], s2 [P,2])."""
    import jax
    r = _get_runner_percore()
    devs, pool = r["devs"], r["pool"]
    oidx = {name: i for i, name in enumerate(r["out_names"])}

    def work(c):
        x8, xs = _quant_x_core(x[c])
        x_c = jax.device_put(x8, devs[c])
        xs_c = jax.device_put(xs, devs[c])
        args = []
        for name in r["in_names"]:
            if name == "x":
                args.append(x_c)
            elif name == "xs":
                args.append(xs_c)
            else:
                args.append(r["wdev"][c][name])
        outs = r["compiled"][c](*args)
        for o in outs:
            try:
                o.copy_to_host_async()
            except Exception:
                pass
        u8t = np.asarray(outs[oidx["out"]])
        tstat = np.asarray(outs[oidx["tstat"]])
        # raw uint8 moments here (cheap; overlaps the other cores'
        # still-running downloads); dequantization is folded into the
        # final BN affine in kernel()
        uf = u8t.astype(np.float32)              # [P, 2, N]
        s0 = uf.sum(axis=2)                      # [P, 2]  sum(u)
        s2 = np.einsum('pon,pon->po', uf, uf)    # [P, 2]  sum(u^2)
        return uf, tstat, s0, s2

    futs = [pool.submit(work, c) for c in range(NCORES)]
    return [f.result() for f in futs]


def _run_spmd_fallback(x):
    from concourse.bass_utils import run_bass_kernel_spmd
    nc = _get_nc()
    wh = _host_weights()
    in_maps = []
    for c in range(NCORES):
        x8, xs = _quant_x_core(x[c])
        in_maps.append({"x": x8, "xs": xs, **wh})
    _CACHE["last_in_maps"] = in_maps
    res = run_bass_kernel_spmd(nc, in_maps, core_ids=list(range(NCORES)))
    _CACHE["last_res"] = res
    return (np.stack([res.results[c]["out"] for c in range(NCORES)]),
            np.stack([res.results[c]["tstat"] for c in range(NCORES)]))


def kernel(x, wq, wv, bv, wt, bt, gamma, beta):
    import hashlib
    x = np.asarray(x, dtype=np.float32)
    wfp = hashlib.md5(
        np.asarray(wq, np.float32).tobytes()
        + np.asarray(wv, np.float32).tobytes()
        + np.asarray(wt, np.float32).tobytes()).hexdigest()
    if _CACHE.get("wfp") != wfp:
        # weights changed (or first call): drop host + device weight caches
        _CACHE.pop("weights", None)
        _CACHE["_raw_w"] = (wq, wv, wt)
        _CACHE["wfp"] = wfp
        rpc = _CACHE.get("runner_pc")
        if rpc is not None:
            import jax
            wh = _host_weights()
            for c in range(NCORES):
                rpc["wdev"][c] = {
                    name: jax.device_put(wh[name], rpc["devs"][c])
                    for name in rpc["in_names"] if name not in ("x", "xs")}

    parts = None
    if _CACHE.get("_pc_fail_count", 0) < 2:
        try:
            parts = _run_percore(x)
            _CACHE["_pc_fail_count"] = 0
        except Exception as e:
            import traceback
            _CACHE["_pc_fail_count"] = _CACHE.get("_pc_fail_count", 0) + 1
            print("percore path failed, falling back:", repr(e)[:300],
                  file=sys.stderr)
            traceback.print_exc()
    if parts is None:
        u8t, tstat = _run_spmd_fallback(x)
        parts = []
        for b in range(B):
            uf = u8t[b].astype(np.float32)
            parts.append((uf, tstat[b], uf.sum(axis=2),
                          np.einsum('pon,pon->po', uf, uf)))

    # ---- host tail: global BN stats via the quantization affine identity
    # t = sc*u + off  =>  sum(t) = sc*sum(u) + N*off
    #                     sum(t^2) = sc^2*sum(u^2) + 2*sc*off*sum(u) + N*off^2
    from concurrent.futures import ThreadPoolExecutor
    pool = _CACHE.setdefault("_tail_pool", ThreadPoolExecutor(NCORES))
    M = B * N
    s1 = np.zeros((P, 2), np.float64)
    s2 = np.zeros((P, 2), np.float64)
    offs, scs = [], []
    for uf, tstat, s0c, s2c in parts:
        off = tstat[:, 0::2].astype(np.float64)          # [P, 2]
        sc = tstat[:, 1::2].astype(np.float64)
        s0c = s0c.astype(np.float64)
        s1 += sc * s0c + N * off
        s2 += sc * sc * s2c.astype(np.float64) + 2.0 * sc * off * s0c + N * off * off
        offs.append(off)
        scs.append(sc)
    mean = s1 / M
    var = s2 / M - mean * mean
    rstd = 1.0 / np.sqrt(var + BN_EPS)
    g2 = np.asarray(gamma, np.float64).reshape(2, P).T   # [P, 2]
    b2 = np.asarray(beta, np.float64).reshape(2, P).T
    A = g2 * rstd
    Bc = b2 - mean * A
    out = np.empty((B, C, N), np.float32)

    def _apply(b):
        # bn = A*t + Bc = (A*sc)*u + (A*off + Bc): fold dequant into BN
        uf = parts[b][0]
        A2 = (A * scs[b])[:, :, None].astype(np.float32)
        B2 = (A * offs[b] + Bc)[:, :, None].astype(np.float32)
        uf *= A2
        uf += B2
        np.maximum(uf, 0.0, out=uf)
        # fused transpose + residual add: out[b] = relu(bn) + x[b]
        np.add(uf.transpose(1, 0, 2), x[b].reshape(2, P, N),
               out=out[b].reshape(2, P, N))

    list(pool.map(_apply, range(B)))
    return out


# revision 27
# speedup vs baseline: 1.0456x; 1.0157x over previous
"""L2-distance attention layer on 8 Trainium2 NeuronCores.

Sharding: data-parallel over batch B=8 (one sample per core); weights
replicated. The per-core Bass kernel computes t = wt @ (softmax(-l2) @ v)
and returns it; BatchNorm (global stats over B and N), the ReLU and the
residual add run on the host in fp32. Moving BN to the host removes the
on-device AllReduce, which makes the 8 cores fully independent — each
core's upload/compute/download pipelines through the device link
concurrently with the others.

Wire format (link is the bottleneck at ~40 MB/s):
  - x up: int8 with a per-(batch, point) column scale (scale = colmax/127),
    dequantized on device into fp16.
  - t down: uint8 with a per-(core, channel) affine (offset = mu - QR*sd,
    scale = 2*QR*sd / 254), computed on device from local channel stats.
    The device also ships its exact per-channel sum(t)/sum(t^2) in tstat,
    so global BN coefficients are ready as soon as the 8 tiny tstat
    tensors land — each core's BN-apply then runs inside its own fetch
    thread, overlapped with the other cores' still-streaming downloads.
  - weights: fp16 (wq, wv) / fp32 (wt), uploaded once and cached on device.

Math notes (validated against the reference):
  - The L2 distance matrix is symmetric with exactly-zero diagonal, so
    softmax(-l2) needs no row-max subtraction (row max is always 0).
  - d2 is computed in ONE matmul per tile via augmented vectors:
    [q; sq; 1]^T [-2q; 1; sq] -> sq_j - 2 q_j.q_i + sq_i.
  - attention rows sum to 1, so bv shifts t by a per-channel constant;
    per-channel constants cancel inside train-mode BatchNorm, as does bt.
"""
import sys
sys.path.insert(0, '/opt/trn_rl_repo')
import numpy as np

B, C, N = 8, 256, 2048
C4 = C // 4          # 64
P = 128
JC = N // P          # 16 j-chunks
NB = N // 512        # 4 i-blocks
NCORES = 8
BN_EPS = 1e-5
QR = 4.0             # t quantization half-range in channel sigmas (MSE-optimal
                     # clip point for 8-bit uniform quantization of a Gaussian)

_CACHE = {}


def _build():
    import concourse.bass as bass
    import concourse.tile as tile
    from concourse import bacc, mybir
    f32 = mybir.dt.float32
    f16 = mybir.dt.float16
    u8 = mybir.dt.uint8

    nc = bacc.Bacc("TRN2", target_bir_lowering=False, debug=False,
                   num_devices=1)
    x_d = nc.dram_tensor("x", [P, 2, N], mybir.dt.int8, kind="ExternalInput")
    xs_d = nc.dram_tensor("xs", [N], f16, kind="ExternalInput")
    wq_d = nc.dram_tensor("wqT", [P, 2, C4], f16, kind="ExternalInput")
    wv_d = nc.dram_tensor("wvT", [P, 2, C], f16, kind="ExternalInput")
    wt_d = nc.dram_tensor("wtT", [P, 2, C], f32, kind="ExternalInput")
    eye_d = nc.dram_tensor("eyem", [P, P], u8, kind="ExternalInput")
    out_d = nc.dram_tensor("out", [P, 2, N], u8, kind="ExternalOutput")
    # tstat columns: [off0, sc0, off1, sc1, s1_0, s1_1, s2_0, s2_1]
    ts_d = nc.dram_tensor("tstat", [P, 8], f32, kind="ExternalOutput")

    AF = mybir.ActivationFunctionType
    OP = mybir.AluOpType

    with tile.TileContext(nc) as tc:
        with tc.tile_pool(name="perm", bufs=1) as perm, \
             tc.tile_pool(name="big", bufs=1) as bigp, \
             tc.tile_pool(name="dram", bufs=1, space="DRAM") as dram:
            # ---- permanent small tiles
            xw = perm.tile([P, 2, N], f16)
            wq = perm.tile([P, 2, C4], f16)
            nc.sync.dma_start(wq[:], wq_d.ap())
            wv = perm.tile([P, 2, C], f16)
            nc.sync.dma_start(wv[:], wv_d.ap())
            wt = perm.tile([P, 2, C], f32)
            nc.sync.dma_start(wt[:], wt_d.ap())
            eye = perm.tile([P, P], u8)
            nc.sync.dma_start(eye[:], eye_d.ap())
            zer = perm.tile([P, P], f32)
            nc.vector.memset(zer[:], 0.0)
            ones64 = perm.tile([C4, 1], f32)
            nc.vector.memset(ones64[:], 1.0)
            vT = perm.tile([P, JC, C], f32)
            dencol = perm.tile([P, JC], f32)
            rep = perm.tile([P, N], f32)
            l2big = bigp.tile([P, JC, N], f32)   # 8 KB/part * 16 = 128 KB/part
            xr = perm.tile([P, 2, N], f32)

            # ---- dequantize x: xw = f16(x8) * colscale
            _xpool = tc.tile_pool(name="xq", bufs=1)
            xqp = _xpool.__enter__()
            x8 = xqp.tile([P, 2, N], mybir.dt.int8)
            nc.sync.dma_start(x8[:], x_d.ap())
            xsr = xqp.tile([P, N], f16)
            xs_ap = xs_d.ap()
            bxs = bass.AP(tensor=xs_ap.tensor, offset=xs_ap.offset,
                          ap=[[0, P], [1, N]])
            nc.sync.dma_start(xsr[:], bxs)
            for oc in range(2):
                nc.vector.tensor_copy(out=xw[:, oc, :], in_=x8[:, oc, :])
                nc.vector.tensor_tensor(out=xw[:, oc, :], in0=xw[:, oc, :],
                                        in1=xsr[:], op=OP.mult)
            _xpool.__exit__(None, None, None)

            # ---- setup: q, sq, A/B bases, vT
            _ABpool = tc.tile_pool(name="ab", bufs=1)
            abp = _ABpool.__enter__()
            At = abp.tile([P, N], f32, tag="A", name="At")
            Bt = abp.tile([P, N], f32, tag="B", name="Bt")
            with tc.tile_pool(name="ps_set", bufs=2, space="PSUM") as pss:
                nc.vector.memset(At[:], 0.0)
                nc.vector.memset(Bt[:], 0.0)
                for nb in range(NB):
                    pq = pss.tile([C4, 512], f32, tag="pq")
                    nc.tensor.matmul(pq[:], lhsT=wq[:, 0, :],
                                     rhs=xw[:, 0, nb * 512:(nb + 1) * 512],
                                     start=True, stop=False)
                    nc.tensor.matmul(pq[:], lhsT=wq[:, 1, :],
                                     rhs=xw[:, 1, nb * 512:(nb + 1) * 512],
                                     start=False, stop=True)
                    nc.vector.tensor_copy(out=At[0:C4, nb * 512:(nb + 1) * 512],
                                          in_=pq[:])
                # q^2 into B rows 0:64 (scratch), then sq row
                nc.vector.tensor_tensor(out=Bt[0:C4, :], in0=At[0:C4, :],
                                        in1=At[0:C4, :], op=OP.mult)
                for nb in range(NB):
                    psq = pss.tile([1, 512], f32, tag="psq")
                    nc.tensor.matmul(psq[:],
                                     lhsT=ones64[:], rhs=Bt[0:C4, nb * 512:(nb + 1) * 512],
                                     start=True, stop=True)
                    nc.vector.tensor_copy(out=At[C4:C4 + 1, nb * 512:(nb + 1) * 512], in_=psq[:])
                    nc.vector.tensor_copy(out=Bt[96:97, nb * 512:(nb + 1) * 512], in_=psq[:])
                # overwrite B rows 0:64 with -2q (after sq matmuls read them)
                nc.vector.tensor_scalar(out=Bt[0:C4, :], in0=At[0:C4, :],
                                        scalar1=-2.0, scalar2=0.0,
                                        op0=OP.mult, op1=OP.add)
                nc.vector.memset(At[96:97, :], 1.0)
                nc.vector.memset(Bt[C4:C4 + 1, :], 1.0)
                # vT
                for jc in range(JC):
                    pv = pss.tile([P, C], f32, tag="pv")
                    nc.tensor.matmul(pv[:], lhsT=xw[:, 0, jc * P:(jc + 1) * P],
                                     rhs=wv[:, 0, :], start=True, stop=False)
                    nc.tensor.matmul(pv[:], lhsT=xw[:, 1, jc * P:(jc + 1) * P],
                                     rhs=wv[:, 1, :], start=False, stop=True)
                    nc.vector.tensor_copy(out=vT[:, jc, :], in_=pv[:])

            # ---- phase A: d2 tiles -> sqrt -> l2big
            with tc.tile_pool(name="ps_d2", bufs=2, space="PSUM") as psd:
                for a in range(JC):
                    pd2 = psd.tile([P, N], f32, tag="d2")
                    for nb in range(NB):
                        nc.tensor.matmul(pd2[:, nb * 512:(nb + 1) * 512],
                                         lhsT=At[:, a * P:(a + 1) * P],
                                         rhs=Bt[:, nb * 512:(nb + 1) * 512],
                                         start=True, stop=True)
                    nc.scalar.activation(l2big[:, a, :], pd2[:], AF.Sqrt)
                    # exact-zero the diagonal block (kills NaN from sqrt(neg))
                    nc.vector.copy_predicated(
                        out=l2big[:, a, a * P:(a + 1) * P],
                        mask=eye[:], data=zer[:])

            _ABpool.__exit__(None, None, None)
            # ---- phase B: exp (+den accum) and attn@v
            with tc.tile_pool(name="post", bufs=1) as postp:
                u8out = postp.tile([P, 2, N], u8)
                tstat = postp.tile([P, 8], f32)
                psav_cm = tc.tile_pool(name="ps_av", bufs=1, space="PSUM")
                psav = psav_cm.__enter__()
                pav = [psav.tile([P, 512], f32, tag=f"av{i}", name=f"pav{i}") for i in range(8)]
                for a in range(JC):
                    Pst = l2big[:, a, :]
                    nc.scalar.activation(Pst, l2big[:, a, :], AF.Exp,
                                         scale=-1.0,
                                         accum_out=dencol[:, a:a + 1])
                    for oc in range(2):
                        for ib in range(NB):
                            nc.tensor.matmul(
                                pav[oc * NB + ib][:],
                                lhsT=vT[:, a, oc * P:(oc + 1) * P],
                                rhs=Pst[:, ib * 512:(ib + 1) * 512],
                                start=(a == 0), stop=(a == JC - 1))

                # ---- denominators -> reciprocal -> broadcast row
                rden = perm.tile([P, JC], f32)
                nc.vector.reciprocal(rden[:], dencol[:])
                dden = dram.tile([N], f32)
                nc.sync.dma_start(dden.rearrange("(a r) -> r a", r=P), rden[:])
                bsrc = bass.AP(tensor=dden.tensor, offset=dden.offset,
                               ap=[[0, P], [1, N]])
                nc.sync.dma_start(rep[:], bsrc)

                # ---- x_r = pav * rep (normalize)
                for oc in range(2):
                    for ib in range(NB):
                        nc.vector.tensor_tensor(
                            out=xr[:, oc, ib * 512:(ib + 1) * 512],
                            in0=pav[oc * NB + ib][:],
                            in1=rep[:, ib * 512:(ib + 1) * 512], op=OP.mult)

                psav_cm.__exit__(None, None, None)
                # ---- t = wtT . xr (in place into xr, with s1 accumulation)
                s1p = [[postp.tile([P, 1], f32, name=f"s1_{o}_{n}", tag=f"s1_{o}_{n}")
                        for n in range(NB)] for o in range(2)]
                with tc.tile_pool(name="ps_t", bufs=2, space="PSUM") as pst:
                    for nb in range(NB):
                        ptl = []
                        for oc2 in range(2):
                            pt = pst.tile([P, 512], f32, tag=f"t{oc2}", name=f"pt{oc2}")
                            nc.tensor.matmul(pt[:], lhsT=wt[:, 0, oc2 * P:(oc2 + 1) * P],
                                             rhs=xr[:, 0, nb * 512:(nb + 1) * 512],
                                             start=True, stop=False)
                            nc.tensor.matmul(pt[:], lhsT=wt[:, 1, oc2 * P:(oc2 + 1) * P],
                                             rhs=xr[:, 1, nb * 512:(nb + 1) * 512],
                                             start=False, stop=True)
                            ptl.append(pt)
                        for oc2 in range(2):
                            nc.vector.tensor_scalar(
                                out=xr[:, oc2, nb * 512:(nb + 1) * 512],
                                in0=ptl[oc2][:], scalar1=1.0, scalar2=0.0,
                                op0=OP.mult, op1=OP.add,
                                accum_out=s1p[oc2][nb][:])

                # ---- per-channel stats: s1, s2 -> mu, sd -> offset/scale
                st = postp.tile([P, 8], f32)
                for oc2 in range(2):
                    nc.vector.tensor_tensor(out=st[:, oc2:oc2 + 1],
                                            in0=s1p[oc2][0][:], in1=s1p[oc2][1][:],
                                            op=OP.add)
                    nc.vector.tensor_tensor(out=st[:, oc2:oc2 + 1],
                                            in0=st[:, oc2:oc2 + 1], in1=s1p[oc2][2][:],
                                            op=OP.add)
                    nc.vector.tensor_tensor(out=st[:, oc2:oc2 + 1],
                                            in0=st[:, oc2:oc2 + 1], in1=s1p[oc2][3][:],
                                            op=OP.add)
                    # s2 via accumulating square pass (scratch into l2big)
                    nc.vector.scalar_tensor_tensor(
                        out=l2big[:, oc2, :], in0=xr[:, oc2, :], scalar=1.0,
                        in1=xr[:, oc2, :], op0=OP.mult, op1=OP.mult,
                        accum_out=st[:, 2 + oc2:3 + oc2])

                INV_N = 1.0 / N
                mu = postp.tile([P, 2], f32)
                sd = postp.tile([P, 2], f32)
                isc = postp.tile([P, 2], f32)
                qb = postp.tile([P, 2], f32)
                epst = postp.tile([P, 1], f32)
                nc.vector.memset(epst[:], 1e-12)
                for oc2 in range(2):
                    nc.vector.tensor_scalar(out=mu[:, oc2:oc2 + 1],
                                            in0=st[:, oc2:oc2 + 1],
                                            scalar1=INV_N, scalar2=0.0,
                                            op0=OP.mult, op1=OP.add)
                    # var = s2/N - mu^2
                    nc.vector.scalar_tensor_tensor(
                        out=sd[:, oc2:oc2 + 1], in0=mu[:, oc2:oc2 + 1],
                        scalar=-1.0, in1=mu[:, oc2:oc2 + 1],
                        op0=OP.mult, op1=OP.mult)
                    nc.vector.scalar_tensor_tensor(
                        out=sd[:, oc2:oc2 + 1], in0=st[:, 2 + oc2:3 + oc2],
                        scalar=INV_N, in1=sd[:, oc2:oc2 + 1],
                        op0=OP.mult, op1=OP.subtract)
                    # sd = sqrt(var) (+tiny eps to avoid 0)
                    nc.scalar.activation(sd[:, oc2:oc2 + 1], sd[:, oc2:oc2 + 1],
                                         AF.Sqrt, bias=epst[:])
                    # isc = 254/(2*QR*sd); offset = mu - QR*sd
                    nc.vector.tensor_scalar(out=isc[:, oc2:oc2 + 1],
                                            in0=sd[:, oc2:oc2 + 1],
                                            scalar1=(2.0 * QR) / 254.0, scalar2=0.0,
                                            op0=OP.mult, op1=OP.add)
                    nc.vector.reciprocal(isc[:, oc2:oc2 + 1], isc[:, oc2:oc2 + 1])
                    # tstat columns: [off0, sc0, off1, sc1]
                    nc.vector.scalar_tensor_tensor(
                        out=tstat[:, 2 * oc2:2 * oc2 + 1], in0=sd[:, oc2:oc2 + 1],
                        scalar=-QR, in1=mu[:, oc2:oc2 + 1],
                        op0=OP.mult, op1=OP.add)
                    nc.vector.tensor_scalar(out=tstat[:, 2 * oc2 + 1:2 * oc2 + 2],
                                            in0=sd[:, oc2:oc2 + 1],
                                            scalar1=(2.0 * QR) / 254.0, scalar2=0.0,
                                            op0=OP.mult, op1=OP.add)
                    # qb = -off*isc  (u = round(t*isc + qb): the DVE
                    # f32->uint8 cast rounds to nearest on this HW, so no
                    # +0.5 truncation compensation — adding it would bias
                    # t_hat by half a step, which exact-t stats don't absorb)
                    nc.vector.tensor_tensor(out=qb[:, oc2:oc2 + 1],
                                            in0=tstat[:, 2 * oc2:2 * oc2 + 1],
                                            in1=isc[:, oc2:oc2 + 1], op=OP.mult)
                    nc.vector.tensor_scalar(out=qb[:, oc2:oc2 + 1],
                                            in0=qb[:, oc2:oc2 + 1],
                                            scalar1=-1.0, scalar2=0.0,
                                            op0=OP.mult, op1=OP.add)

                # export the exact local t sums for host-side BN stats
                nc.vector.tensor_scalar(out=tstat[:, 4:6], in0=st[:, 0:2],
                                        scalar1=1.0, scalar2=0.0,
                                        op0=OP.mult, op1=OP.add)
                nc.vector.tensor_scalar(out=tstat[:, 6:8], in0=st[:, 2:4],
                                        scalar1=1.0, scalar2=0.0,
                                        op0=OP.mult, op1=OP.add)

                # ---- quantize: u8 = min(relu(t*isc + qb), 254.99) truncated
                for oc2 in range(2):
                    z = l2big[:, 4 + oc2, :]
                    nc.scalar.activation(z, xr[:, oc2, :], AF.Relu,
                                         scale=isc[:, oc2:oc2 + 1],
                                         bias=qb[:, oc2:oc2 + 1])
                    nc.vector.tensor_scalar(out=u8out[:, oc2, :], in0=z,
                                            scalar1=254.99, scalar2=0.0,
                                            op0=OP.min, op1=OP.add)
                nc.sync.dma_start(out_d.ap(), u8out[:])
                nc.sync.dma_start(ts_d.ap(), tstat[:])

    nc.compile()
    return nc


def _get_nc():
    if "nc" not in _CACHE:
        _CACHE["nc"] = _build()
    return _CACHE["nc"]


def _host_weights():
    # per-core weight arrays (fp16 wire for wq/wv, fp32 for wt)
    if "weights" in _CACHE:
        return _CACHE["weights"]
    wq, wv, wt = _CACHE["_raw_w"]
    wqT = np.ascontiguousarray(
        np.asarray(wq, np.float32).T.reshape(2, P, C4).transpose(1, 0, 2)).astype(np.float16)
    wvT = np.ascontiguousarray(
        np.asarray(wv, np.float32).T.reshape(2, P, C).transpose(1, 0, 2)).astype(np.float16)
    wtT = np.ascontiguousarray(
        np.asarray(wt, np.float32).T.reshape(2, P, C).transpose(1, 0, 2))
    eyem = np.eye(P, dtype=np.uint8)
    _CACHE["weights"] = {"wqT": wqT, "wvT": wvT, "wtT": wtT, "eyem": eyem}
    return _CACHE["weights"]


def _quant_x_core(xb):
    """xb: [C, N] f32 for one batch -> (x8 [P,2,N] int8, xs [N] f16)."""
    colmax = np.abs(xb).max(axis=0)
    np.maximum(colmax, 1e-12, out=colmax)
    xs = (colmax / 127.0).astype(np.float16)
    inv = 127.0 / colmax
    xq = np.rint(xb.reshape(2, P, N).transpose(1, 0, 2) * inv)
    np.clip(xq, -127, 127, out=xq)
    return xq.astype(np.int8), xs


def _io_names(nc):
    from concourse import mybir
    import jax
    in_names, out_names, out_avals = [], [], []
    for alloc in nc.m.functions[0].allocations:
        if not isinstance(alloc, mybir.MemoryLocationSet):
            continue
        name = alloc.memorylocations[0].name
        if alloc.kind == "ExternalInput":
            in_names.append(name)
        elif alloc.kind == "ExternalOutput":
            out_names.append(name)
            out_avals.append(jax.core.ShapedArray(
                tuple(alloc.tensor_shape), mybir.dt.np(alloc.dtype)))
    return in_names, out_names, out_avals


def _get_runner_percore():
    """Per-device AOT executables: upload/compute/download pipeline per core."""
    if "runner_pc" in _CACHE:
        return _CACHE["runner_pc"]
    import jax
    from concurrent.futures import ThreadPoolExecutor
    from concourse import bass2jax
    bass2jax.install_neuronx_cc_hook()

    nc = _get_nc()
    in_names, out_names, out_avals = _io_names(nc)
    partition_name = nc.partition_id_tensor.name if nc.partition_id_tensor else None
    if partition_name is not None and partition_name in in_names:
        in_names.remove(partition_name)
    all_in_names = list(in_names)
    if partition_name is not None:
        all_in_names.append(partition_name)

    devs = jax.devices()[:NCORES]

    def _body(*args):
        operands = list(args)
        if partition_name is not None:
            operands.append(bass2jax.partition_id_tensor())
        outs = bass2jax._bass_exec_p.bind(
            *operands,
            out_avals=tuple(out_avals),
            in_names=tuple(all_in_names),
            out_names=tuple(out_names),
            lowering_input_output_aliases=(),
            sim_require_finite=True,
            sim_require_nnan=True,
            nc=nc,
        )
        return tuple(outs)

    wh = _host_weights()
    in_specs_np = {"x": np.zeros((P, 2, N), np.int8),
                   "xs": np.zeros((N,), np.float16)}

    def _compile_core(c):
        specs = []
        for name in in_names:
            arr = in_specs_np.get(name, wh.get(name))
            specs.append(jax.ShapeDtypeStruct(
                arr.shape, arr.dtype,
                sharding=jax.sharding.SingleDeviceSharding(devs[c])))
        try:
            comp = bass2jax.fast_dispatch_compile(
                lambda: jax.jit(_body, keep_unused=True).lower(*specs).compile())
        except Exception:
            comp = jax.jit(_body, keep_unused=True).lower(*specs).compile()
        wd = {name: jax.device_put(wh[name], devs[c])
              for name in in_names if name not in ("x", "xs")}
        return comp, wd

    cpool = ThreadPoolExecutor(NCORES)
    futs = [cpool.submit(_compile_core, c) for c in range(NCORES)]
    results = [f.result() for f in futs]
    compiled = [r0 for r0, _ in results]
    wdev = [r1 for _, r1 in results]

    runner = {
        "compiled": compiled, "devs": devs, "wdev": wdev,
        "in_names": in_names, "out_names": out_names,
        "pool": ThreadPoolExecutor(NCORES),
    }
    _CACHE["runner_pc"] = runner
    return runner


def _bn_coeffs(s1, s2, gamma, beta):
    """Global BN stats (float64 [P,2] sums) -> affine A, Bc."""
    M = B * N
    mean = s1 / M
    var = s2 / M - mean * mean
    rstd = 1.0 / np.sqrt(var + BN_EPS)
    g2 = np.asarray(gamma, np.float64).reshape(2, P).T   # [P, 2]
    b2 = np.asarray(beta, np.float64).reshape(2, P).T
    A = g2 * rstd
    Bc = b2 - mean * A
    return A, Bc


def _apply_core(uf, tst, A, Bc, xb, ob):
    """In-thread: bn = A*t + Bc with t = sc*u + off folded in, relu, +x."""
    off = tst[:, [0, 2]].astype(np.float64)              # [P, 2]
    sc = tst[:, [1, 3]].astype(np.float64)
    A2 = (A * sc)[:, :, None].astype(np.float32)
    B2 = (A * off + Bc)[:, :, None].astype(np.float32)
    uf *= A2
    uf += B2
    np.maximum(uf, 0.0, out=uf)
    np.add(uf.transpose(1, 0, 2), xb.reshape(2, P, N),
           out=ob.reshape(2, P, N))


def _run_percore(x, gamma, beta, out):
    """x: [B, C, N] f32. Fills out [B, C, N] f32 completely."""
    import jax
    import threading
    r = _get_runner_percore()
    devs, pool = r["devs"], r["pool"]
    oidx = {name: i for i, name in enumerate(r["out_names"])}
    lock = threading.Lock()
    stats = {}
    box = {}
    ev = threading.Event()

    def work(c):
        x8, xs = _quant_x_core(x[c])
        x_c = jax.device_put(x8, devs[c])
        xs_c = jax.device_put(xs, devs[c])
        args = []
        for name in r["in_names"]:
            if name == "x":
                args.append(x_c)
            elif name == "xs":
                args.append(xs_c)
            else:
                args.append(r["wdev"][c][name])
        outs = r["compiled"][c](*args)
        for o in outs:
            try:
                o.copy_to_host_async()
            except Exception:
                pass
        # tstat is tiny and lands well before the 512 KB out tensor: as soon
        # as all 8 are in, global BN coefficients are ready and every core's
        # BN-apply happens inside its own fetch thread (no serial tail).
        tst = np.asarray(outs[oidx["tstat"]])
        with lock:
            stats[c] = tst
            if len(stats) == NCORES:
                s1 = np.zeros((P, 2), np.float64)
                s2 = np.zeros((P, 2), np.float64)
                for k in stats:
                    s1 += stats[k][:, 4:6].astype(np.float64)
                    s2 += stats[k][:, 6:8].astype(np.float64)
                box["A"], box["Bc"] = _bn_coeffs(s1, s2, gamma, beta)
                ev.set()
        u8t = np.asarray(outs[oidx["out"]])
        uf = u8t.astype(np.float32)              # [P, 2, N]
        if not ev.wait(timeout=60.0):
            raise RuntimeError("BN stats barrier timed out")
        _apply_core(uf, tst, box["A"], box["Bc"], x[c], out[c])

    futs = [pool.submit(work, c) for c in range(NCORES)]
    [f.result() for f in futs]


def _run_spmd_fallback(x):
    from concourse.bass_utils import run_bass_kernel_spmd
    nc = _get_nc()
    wh = _host_weights()
    in_maps = []
    for c in range(NCORES):
        x8, xs = _quant_x_core(x[c])
        in_maps.append({"x": x8, "xs": xs, **wh})
    _CACHE["last_in_maps"] = in_maps
    res = run_bass_kernel_spmd(nc, in_maps, core_ids=list(range(NCORES)))
    _CACHE["last_res"] = res
    return (np.stack([res.results[c]["out"] for c in range(NCORES)]),
            np.stack([res.results[c]["tstat"] for c in range(NCORES)]))


def kernel(x, wq, wv, bv, wt, bt, gamma, beta):
    import hashlib
    x = np.asarray(x, dtype=np.float32)
    wfp = hashlib.md5(
        np.asarray(wq, np.float32).tobytes()
        + np.asarray(wv, np.float32).tobytes()
        + np.asarray(wt, np.float32).tobytes()).hexdigest()
    if _CACHE.get("wfp") != wfp:
        # weights changed (or first call): drop host + device weight caches
        _CACHE.pop("weights", None)
        _CACHE["_raw_w"] = (wq, wv, wt)
        _CACHE["wfp"] = wfp
        rpc = _CACHE.get("runner_pc")
        if rpc is not None:
            import jax
            wh = _host_weights()
            for c in range(NCORES):
                rpc["wdev"][c] = {
                    name: jax.device_put(wh[name], rpc["devs"][c])
                    for name in rpc["in_names"] if name not in ("x", "xs")}

    out = np.empty((B, C, N), np.float32)
    done = False
    if _CACHE.get("_pc_fail_count", 0) < 2:
        try:
            _run_percore(x, gamma, beta, out)
            _CACHE["_pc_fail_count"] = 0
            done = True
        except Exception as e:
            import traceback
            _CACHE["_pc_fail_count"] = _CACHE.get("_pc_fail_count", 0) + 1
            print("percore path failed, falling back:", repr(e)[:300],
                  file=sys.stderr)
            traceback.print_exc()
    if not done:
        u8t, tstat = _run_spmd_fallback(x)
        s1 = np.zeros((P, 2), np.float64)
        s2 = np.zeros((P, 2), np.float64)
        for b in range(B):
            s1 += tstat[b][:, 4:6].astype(np.float64)
            s2 += tstat[b][:, 6:8].astype(np.float64)
        A, Bc = _bn_coeffs(s1, s2, gamma, beta)
        for b in range(B):
            _apply_core(u8t[b].astype(np.float32), tstat[b], A, Bc,
                        x[b], out[b])
    return out


# revision 33
# speedup vs baseline: 1.0784x; 1.0314x over previous
"""L2-distance attention layer on 8 Trainium2 NeuronCores.

Sharding: data-parallel over batch B=8 (one sample per core); weights
replicated. The per-core Bass kernel computes t = wt @ (softmax(-l2) @ v)
and returns it; BatchNorm (global stats over B and N), the ReLU and the
residual add run on the host in fp32. Moving BN to the host removes the
on-device AllReduce, which makes the 8 cores fully independent — each
core's upload/compute/download pipelines through the device link
concurrently with the others.

Wire format (link is the bottleneck at ~40 MB/s):
  - x up: int8 with a per-(batch, point) column scale (scale = colmax/127),
    dequantized on device into fp16.
  - t down: uint8 with a per-(core, channel) affine (offset = mu - QR*sd,
    scale = 2*QR*sd / 254), computed on device from local channel stats.
    The device also ships its exact per-channel sum(t)/sum(t^2) in tstat,
    so global BN coefficients are ready as soon as the 8 tiny tstat
    tensors land — each core's BN-apply then runs inside its own fetch
    thread, overlapped with the other cores' still-streaming downloads.
  - weights: fp16 (wq, wv) / fp32 (wt), uploaded once and cached on device.

Math notes (validated against the reference):
  - The L2 distance matrix is symmetric with exactly-zero diagonal, so
    softmax(-l2) needs no row-max subtraction (row max is always 0).
  - d2 is computed in ONE matmul per tile via augmented vectors:
    [q; sq; 1]^T [-2q; 1; sq] -> sq_j - 2 q_j.q_i + sq_i.
  - attention rows sum to 1, so bv shifts t by a per-channel constant;
    per-channel constants cancel inside train-mode BatchNorm, as does bt.
"""
import sys
sys.path.insert(0, '/opt/trn_rl_repo')
import numpy as np

B, C, N = 8, 256, 2048
C4 = C // 4          # 64
P = 128
JC = N // P          # 16 j-chunks
NB = N // 512        # 4 i-blocks
NCORES = 8
BN_EPS = 1e-5
QR = 4.0             # t quantization half-range in channel sigmas (MSE-optimal
                     # clip point for 8-bit uniform quantization of a Gaussian)

_CACHE = {}


def _build():
    import concourse.bass as bass
    import concourse.tile as tile
    from concourse import bacc, mybir
    f32 = mybir.dt.float32
    f16 = mybir.dt.float16
    u8 = mybir.dt.uint8

    nc = bacc.Bacc("TRN2", target_bir_lowering=False, debug=False,
                   num_devices=1)
    x_d = nc.dram_tensor("x", [P, 2, N], mybir.dt.int8, kind="ExternalInput")
    xs_d = nc.dram_tensor("xs", [N], f16, kind="ExternalInput")
    wq_d = nc.dram_tensor("wqT", [P, 2, C4], f16, kind="ExternalInput")
    wv_d = nc.dram_tensor("wvT", [P, 2, C], f16, kind="ExternalInput")
    wt_d = nc.dram_tensor("wtT", [P, 2, C], f32, kind="ExternalInput")
    eye_d = nc.dram_tensor("eyem", [P, P], u8, kind="ExternalInput")
    out_d = nc.dram_tensor("out", [P, 2, N], u8, kind="ExternalOutput")
    # tstat columns: [off0, sc0, off1, sc1, s1_0, s1_1, s2_0, s2_1]
    ts_d = nc.dram_tensor("tstat", [P, 8], f32, kind="ExternalOutput")

    AF = mybir.ActivationFunctionType
    OP = mybir.AluOpType

    with tile.TileContext(nc) as tc:
        with tc.tile_pool(name="perm", bufs=1) as perm, \
             tc.tile_pool(name="big", bufs=1) as bigp, \
             tc.tile_pool(name="dram", bufs=1, space="DRAM") as dram:
            # ---- permanent small tiles
            xw = perm.tile([P, 2, N], f16)
            wq = perm.tile([P, 2, C4], f16)
            nc.sync.dma_start(wq[:], wq_d.ap())
            wv = perm.tile([P, 2, C], f16)
            nc.sync.dma_start(wv[:], wv_d.ap())
            wt = perm.tile([P, 2, C], f32)
            nc.sync.dma_start(wt[:], wt_d.ap())
            eye = perm.tile([P, P], u8)
            nc.sync.dma_start(eye[:], eye_d.ap())
            zer = perm.tile([P, P], f32)
            nc.vector.memset(zer[:], 0.0)
            ones64 = perm.tile([C4, 1], f32)
            nc.vector.memset(ones64[:], 1.0)
            vT = perm.tile([P, JC, C], f32)
            dencol = perm.tile([P, JC], f32)
            rep = perm.tile([P, N], f32)
            l2big = bigp.tile([P, JC, N], f32)   # 8 KB/part * 16 = 128 KB/part
            xr = perm.tile([P, 2, N], f32)

            # ---- dequantize x: xw = f16(x8) * colscale
            _xpool = tc.tile_pool(name="xq", bufs=1)
            xqp = _xpool.__enter__()
            x8 = xqp.tile([P, 2, N], mybir.dt.int8)
            nc.sync.dma_start(x8[:], x_d.ap())
            xsr = xqp.tile([P, N], f16)
            xs_ap = xs_d.ap()
            bxs = bass.AP(tensor=xs_ap.tensor, offset=xs_ap.offset,
                          ap=[[0, P], [1, N]])
            nc.sync.dma_start(xsr[:], bxs)
            for oc in range(2):
                nc.vector.tensor_copy(out=xw[:, oc, :], in_=x8[:, oc, :])
                nc.vector.tensor_tensor(out=xw[:, oc, :], in0=xw[:, oc, :],
                                        in1=xsr[:], op=OP.mult)
            _xpool.__exit__(None, None, None)

            # ---- setup: q, sq, A/B bases, vT
            _ABpool = tc.tile_pool(name="ab", bufs=1)
            abp = _ABpool.__enter__()
            At = abp.tile([P, N], f32, tag="A", name="At")
            Bt = abp.tile([P, N], f32, tag="B", name="Bt")
            with tc.tile_pool(name="ps_set", bufs=2, space="PSUM") as pss:
                nc.vector.memset(At[:], 0.0)
                nc.vector.memset(Bt[:], 0.0)
                for nb in range(NB):
                    pq = pss.tile([C4, 512], f32, tag="pq")
                    nc.tensor.matmul(pq[:], lhsT=wq[:, 0, :],
                                     rhs=xw[:, 0, nb * 512:(nb + 1) * 512],
                                     start=True, stop=False)
                    nc.tensor.matmul(pq[:], lhsT=wq[:, 1, :],
                                     rhs=xw[:, 1, nb * 512:(nb + 1) * 512],
                                     start=False, stop=True)
                    nc.vector.tensor_copy(out=At[0:C4, nb * 512:(nb + 1) * 512],
                                          in_=pq[:])
                # q^2 into B rows 0:64 (scratch), then sq row
                nc.vector.tensor_tensor(out=Bt[0:C4, :], in0=At[0:C4, :],
                                        in1=At[0:C4, :], op=OP.mult)
                for nb in range(NB):
                    psq = pss.tile([1, 512], f32, tag="psq")
                    nc.tensor.matmul(psq[:],
                                     lhsT=ones64[:], rhs=Bt[0:C4, nb * 512:(nb + 1) * 512],
                                     start=True, stop=True)
                    nc.vector.tensor_copy(out=At[C4:C4 + 1, nb * 512:(nb + 1) * 512], in_=psq[:])
                    nc.vector.tensor_copy(out=Bt[96:97, nb * 512:(nb + 1) * 512], in_=psq[:])
                # overwrite B rows 0:64 with -2q (after sq matmuls read them)
                nc.vector.tensor_scalar(out=Bt[0:C4, :], in0=At[0:C4, :],
                                        scalar1=-2.0, scalar2=0.0,
                                        op0=OP.mult, op1=OP.add)
                nc.vector.memset(At[96:97, :], 1.0)
                nc.vector.memset(Bt[C4:C4 + 1, :], 1.0)
                # vT
                for jc in range(JC):
                    pv = pss.tile([P, C], f32, tag="pv")
                    nc.tensor.matmul(pv[:], lhsT=xw[:, 0, jc * P:(jc + 1) * P],
                                     rhs=wv[:, 0, :], start=True, stop=False)
                    nc.tensor.matmul(pv[:], lhsT=xw[:, 1, jc * P:(jc + 1) * P],
                                     rhs=wv[:, 1, :], start=False, stop=True)
                    nc.vector.tensor_copy(out=vT[:, jc, :], in_=pv[:])

            # ---- phase A: d2 tiles -> sqrt -> l2big
            with tc.tile_pool(name="ps_d2", bufs=2, space="PSUM") as psd:
                for a in range(JC):
                    pd2 = psd.tile([P, N], f32, tag="d2")
                    for nb in range(NB):
                        nc.tensor.matmul(pd2[:, nb * 512:(nb + 1) * 512],
                                         lhsT=At[:, a * P:(a + 1) * P],
                                         rhs=Bt[:, nb * 512:(nb + 1) * 512],
                                         start=True, stop=True)
                    nc.scalar.activation(l2big[:, a, :], pd2[:], AF.Sqrt)
                    # exact-zero the diagonal block (kills NaN from sqrt(neg))
                    nc.vector.copy_predicated(
                        out=l2big[:, a, a * P:(a + 1) * P],
                        mask=eye[:], data=zer[:])

            _ABpool.__exit__(None, None, None)
            # ---- phase B: exp (+den accum) and attn@v
            with tc.tile_pool(name="post", bufs=1) as postp:
                u8out = postp.tile([P, 2, N], u8)
                tstat = postp.tile([P, 8], f32)
                psav_cm = tc.tile_pool(name="ps_av", bufs=1, space="PSUM")
                psav = psav_cm.__enter__()
                pav = [psav.tile([P, 512], f32, tag=f"av{i}", name=f"pav{i}") for i in range(8)]
                for a in range(JC):
                    Pst = l2big[:, a, :]
                    nc.scalar.activation(Pst, l2big[:, a, :], AF.Exp,
                                         scale=-1.0,
                                         accum_out=dencol[:, a:a + 1])
                    for oc in range(2):
                        for ib in range(NB):
                            nc.tensor.matmul(
                                pav[oc * NB + ib][:],
                                lhsT=vT[:, a, oc * P:(oc + 1) * P],
                                rhs=Pst[:, ib * 512:(ib + 1) * 512],
                                start=(a == 0), stop=(a == JC - 1))

                # ---- denominators -> reciprocal -> broadcast row
                rden = perm.tile([P, JC], f32)
                nc.vector.reciprocal(rden[:], dencol[:])
                dden = dram.tile([N], f32)
                nc.sync.dma_start(dden.rearrange("(a r) -> r a", r=P), rden[:])
                bsrc = bass.AP(tensor=dden.tensor, offset=dden.offset,
                               ap=[[0, P], [1, N]])
                nc.sync.dma_start(rep[:], bsrc)

                # ---- x_r = pav * rep (normalize)
                for oc in range(2):
                    for ib in range(NB):
                        nc.vector.tensor_tensor(
                            out=xr[:, oc, ib * 512:(ib + 1) * 512],
                            in0=pav[oc * NB + ib][:],
                            in1=rep[:, ib * 512:(ib + 1) * 512], op=OP.mult)

                psav_cm.__exit__(None, None, None)
                # ---- t = wtT . xr (in place into xr, with s1 accumulation)
                s1p = [[postp.tile([P, 1], f32, name=f"s1_{o}_{n}", tag=f"s1_{o}_{n}")
                        for n in range(NB)] for o in range(2)]
                with tc.tile_pool(name="ps_t", bufs=2, space="PSUM") as pst:
                    for nb in range(NB):
                        ptl = []
                        for oc2 in range(2):
                            pt = pst.tile([P, 512], f32, tag=f"t{oc2}", name=f"pt{oc2}")
                            nc.tensor.matmul(pt[:], lhsT=wt[:, 0, oc2 * P:(oc2 + 1) * P],
                                             rhs=xr[:, 0, nb * 512:(nb + 1) * 512],
                                             start=True, stop=False)
                            nc.tensor.matmul(pt[:], lhsT=wt[:, 1, oc2 * P:(oc2 + 1) * P],
                                             rhs=xr[:, 1, nb * 512:(nb + 1) * 512],
                                             start=False, stop=True)
                            ptl.append(pt)
                        for oc2 in range(2):
                            nc.vector.tensor_scalar(
                                out=xr[:, oc2, nb * 512:(nb + 1) * 512],
                                in0=ptl[oc2][:], scalar1=1.0, scalar2=0.0,
                                op0=OP.mult, op1=OP.add,
                                accum_out=s1p[oc2][nb][:])

                # ---- per-channel stats: s1, s2 -> mu, sd -> offset/scale
                st = postp.tile([P, 8], f32)
                for oc2 in range(2):
                    nc.vector.tensor_tensor(out=st[:, oc2:oc2 + 1],
                                            in0=s1p[oc2][0][:], in1=s1p[oc2][1][:],
                                            op=OP.add)
                    nc.vector.tensor_tensor(out=st[:, oc2:oc2 + 1],
                                            in0=st[:, oc2:oc2 + 1], in1=s1p[oc2][2][:],
                                            op=OP.add)
                    nc.vector.tensor_tensor(out=st[:, oc2:oc2 + 1],
                                            in0=st[:, oc2:oc2 + 1], in1=s1p[oc2][3][:],
                                            op=OP.add)
                    # s2 via accumulating square pass (scratch into l2big)
                    nc.vector.scalar_tensor_tensor(
                        out=l2big[:, oc2, :], in0=xr[:, oc2, :], scalar=1.0,
                        in1=xr[:, oc2, :], op0=OP.mult, op1=OP.mult,
                        accum_out=st[:, 2 + oc2:3 + oc2])

                INV_N = 1.0 / N
                mu = postp.tile([P, 2], f32)
                sd = postp.tile([P, 2], f32)
                isc = postp.tile([P, 2], f32)
                qb = postp.tile([P, 2], f32)
                epst = postp.tile([P, 1], f32)
                nc.vector.memset(epst[:], 1e-12)
                for oc2 in range(2):
                    nc.vector.tensor_scalar(out=mu[:, oc2:oc2 + 1],
                                            in0=st[:, oc2:oc2 + 1],
                                            scalar1=INV_N, scalar2=0.0,
                                            op0=OP.mult, op1=OP.add)
                    # var = s2/N - mu^2
                    nc.vector.scalar_tensor_tensor(
                        out=sd[:, oc2:oc2 + 1], in0=mu[:, oc2:oc2 + 1],
                        scalar=-1.0, in1=mu[:, oc2:oc2 + 1],
                        op0=OP.mult, op1=OP.mult)
                    nc.vector.scalar_tensor_tensor(
                        out=sd[:, oc2:oc2 + 1], in0=st[:, 2 + oc2:3 + oc2],
                        scalar=INV_N, in1=sd[:, oc2:oc2 + 1],
                        op0=OP.mult, op1=OP.subtract)
                    # sd = sqrt(var) (+tiny eps to avoid 0)
                    nc.scalar.activation(sd[:, oc2:oc2 + 1], sd[:, oc2:oc2 + 1],
                                         AF.Sqrt, bias=epst[:])
                    # isc = 254/(2*QR*sd); offset = mu - QR*sd
                    nc.vector.tensor_scalar(out=isc[:, oc2:oc2 + 1],
                                            in0=sd[:, oc2:oc2 + 1],
                                            scalar1=(2.0 * QR) / 254.0, scalar2=0.0,
                                            op0=OP.mult, op1=OP.add)
                    nc.vector.reciprocal(isc[:, oc2:oc2 + 1], isc[:, oc2:oc2 + 1])
                    # tstat columns: [off0, sc0, off1, sc1]
                    nc.vector.scalar_tensor_tensor(
                        out=tstat[:, 2 * oc2:2 * oc2 + 1], in0=sd[:, oc2:oc2 + 1],
                        scalar=-QR, in1=mu[:, oc2:oc2 + 1],
                        op0=OP.mult, op1=OP.add)
                    nc.vector.tensor_scalar(out=tstat[:, 2 * oc2 + 1:2 * oc2 + 2],
                                            in0=sd[:, oc2:oc2 + 1],
                                            scalar1=(2.0 * QR) / 254.0, scalar2=0.0,
                                            op0=OP.mult, op1=OP.add)
                    # qb = -off*isc  (u = round(t*isc + qb): the DVE
                    # f32->uint8 cast rounds to nearest on this HW, so no
                    # +0.5 truncation compensation — adding it would bias
                    # t_hat by half a step, which exact-t stats don't absorb)
                    nc.vector.tensor_tensor(out=qb[:, oc2:oc2 + 1],
                                            in0=tstat[:, 2 * oc2:2 * oc2 + 1],
                                            in1=isc[:, oc2:oc2 + 1], op=OP.mult)
                    nc.vector.tensor_scalar(out=qb[:, oc2:oc2 + 1],
                                            in0=qb[:, oc2:oc2 + 1],
                                            scalar1=-1.0, scalar2=0.0,
                                            op0=OP.mult, op1=OP.add)

                # export the exact local t sums for host-side BN stats
                nc.vector.tensor_scalar(out=tstat[:, 4:6], in0=st[:, 0:2],
                                        scalar1=1.0, scalar2=0.0,
                                        op0=OP.mult, op1=OP.add)
                nc.vector.tensor_scalar(out=tstat[:, 6:8], in0=st[:, 2:4],
                                        scalar1=1.0, scalar2=0.0,
                                        op0=OP.mult, op1=OP.add)

                # ---- quantize: u8 = min(relu(t*isc + qb), 254.99) truncated
                for oc2 in range(2):
                    z = l2big[:, 4 + oc2, :]
                    nc.scalar.activation(z, xr[:, oc2, :], AF.Relu,
                                         scale=isc[:, oc2:oc2 + 1],
                                         bias=qb[:, oc2:oc2 + 1])
                    nc.vector.tensor_scalar(out=u8out[:, oc2, :], in0=z,
                                            scalar1=254.99, scalar2=0.0,
                                            op0=OP.min, op1=OP.add)
                nc.sync.dma_start(out_d.ap(), u8out[:])
                nc.sync.dma_start(ts_d.ap(), tstat[:])

    nc.compile()
    return nc


def _get_nc():
    if "nc" not in _CACHE:
        _CACHE["nc"] = _build()
    return _CACHE["nc"]


def _host_weights():
    # per-core weight arrays (fp16 wire for wq/wv, fp32 for wt)
    if "weights" in _CACHE:
        return _CACHE["weights"]
    wq, wv, wt = _CACHE["_raw_w"]
    wqT = np.ascontiguousarray(
        np.asarray(wq, np.float32).T.reshape(2, P, C4).transpose(1, 0, 2)).astype(np.float16)
    wvT = np.ascontiguousarray(
        np.asarray(wv, np.float32).T.reshape(2, P, C).transpose(1, 0, 2)).astype(np.float16)
    wtT = np.ascontiguousarray(
        np.asarray(wt, np.float32).T.reshape(2, P, C).transpose(1, 0, 2))
    eyem = np.eye(P, dtype=np.uint8)
    _CACHE["weights"] = {"wqT": wqT, "wvT": wvT, "wtT": wtT, "eyem": eyem}
    return _CACHE["weights"]


def _quant_x_core(xb, c=0):
    """xb: [C, N] f32 for one batch -> (x8 [P,2,N] int8, xs [N] f16).

    Uses per-core cached scratch buffers: fresh 2 MB allocations page-fault
    on every call, which is real time on this 1-CPU container."""
    bufs = _CACHE.setdefault("_qbuf", {})
    if c not in bufs:
        bufs[c] = (np.empty((P, 2, N), np.float32), np.empty((P, 2, N), np.int8),
                   np.empty(N, np.float32), np.empty(N, np.float32))
    f32b, i8b, mx, mn = bufs[c]
    np.max(xb, axis=0, out=mx)
    np.min(xb, axis=0, out=mn)
    np.negative(mn, out=mn)
    np.maximum(mx, mn, out=mx)          # colmax = max(|xb|) without a temp
    np.maximum(mx, 1e-12, out=mx)
    xs = (mx / 127.0).astype(np.float16)
    np.divide(127.0, mx, out=mn)        # inv scale
    np.multiply(xb.reshape(2, P, N).transpose(1, 0, 2), mn, out=f32b)
    np.rint(f32b, out=f32b)
    np.clip(f32b, -127, 127, out=f32b)
    np.copyto(i8b, f32b, casting='unsafe')
    return i8b, xs


def _io_names(nc):
    from concourse import mybir
    import jax
    in_names, out_names, out_avals = [], [], []
    for alloc in nc.m.functions[0].allocations:
        if not isinstance(alloc, mybir.MemoryLocationSet):
            continue
        name = alloc.memorylocations[0].name
        if alloc.kind == "ExternalInput":
            in_names.append(name)
        elif alloc.kind == "ExternalOutput":
            out_names.append(name)
            out_avals.append(jax.core.ShapedArray(
                tuple(alloc.tensor_shape), mybir.dt.np(alloc.dtype)))
    return in_names, out_names, out_avals


def _get_runner_percore():
    """Per-device AOT executables: upload/compute/download pipeline per core."""
    if "runner_pc" in _CACHE:
        return _CACHE["runner_pc"]
    import jax
    from concurrent.futures import ThreadPoolExecutor
    from concourse import bass2jax
    bass2jax.install_neuronx_cc_hook()

    nc = _get_nc()
    in_names, out_names, out_avals = _io_names(nc)
    partition_name = nc.partition_id_tensor.name if nc.partition_id_tensor else None
    if partition_name is not None and partition_name in in_names:
        in_names.remove(partition_name)
    all_in_names = list(in_names)
    if partition_name is not None:
        all_in_names.append(partition_name)

    devs = jax.devices()[:NCORES]

    def _body(*args):
        operands = list(args)
        if partition_name is not None:
            operands.append(bass2jax.partition_id_tensor())
        outs = bass2jax._bass_exec_p.bind(
            *operands,
            out_avals=tuple(out_avals),
            in_names=tuple(all_in_names),
            out_names=tuple(out_names),
            lowering_input_output_aliases=(),
            sim_require_finite=True,
            sim_require_nnan=True,
            nc=nc,
        )
        return tuple(outs)

    wh = _host_weights()
    in_specs_np = {"x": np.zeros((P, 2, N), np.int8),
                   "xs": np.zeros((N,), np.float16)}

    def _compile_core(c):
        specs = []
        for name in in_names:
            arr = in_specs_np.get(name, wh.get(name))
            specs.append(jax.ShapeDtypeStruct(
                arr.shape, arr.dtype,
                sharding=jax.sharding.SingleDeviceSharding(devs[c])))
        try:
            comp = bass2jax.fast_dispatch_compile(
                lambda: jax.jit(_body, keep_unused=True).lower(*specs).compile())
        except Exception:
            comp = jax.jit(_body, keep_unused=True).lower(*specs).compile()
        wd = {name: jax.device_put(wh[name], devs[c])
              for name in in_names if name not in ("x", "xs")}
        return comp, wd

    cpool = ThreadPoolExecutor(NCORES)
    futs = [cpool.submit(_compile_core, c) for c in range(NCORES)]
    results = [f.result() for f in futs]
    compiled = [r0 for r0, _ in results]
    wdev = [r1 for _, r1 in results]

    runner = {
        "compiled": compiled, "devs": devs, "wdev": wdev,
        "in_names": in_names, "out_names": out_names,
        "pool": ThreadPoolExecutor(NCORES),
    }
    _CACHE["runner_pc"] = runner
    return runner


def _bn_coeffs(s1, s2, gamma, beta):
    """Global BN stats (float64 [P,2] sums) -> affine A, Bc."""
    M = B * N
    mean = s1 / M
    var = s2 / M - mean * mean
    rstd = 1.0 / np.sqrt(var + BN_EPS)
    g2 = np.asarray(gamma, np.float64).reshape(2, P).T   # [P, 2]
    b2 = np.asarray(beta, np.float64).reshape(2, P).T
    A = g2 * rstd
    Bc = b2 - mean * A
    return A, Bc


def _apply_core(u8t, tst, A, Bc, xb, ob, c=0):
    """In-thread: bn = A*t + Bc with t = sc*u + off folded in, relu, +x.
    The uint8->f32 cast fuses into the affine multiply (one pass)."""
    bufs = _CACHE.setdefault("_abuf", {})
    if c not in bufs:
        bufs[c] = np.empty((P, 2, N), np.float32)
    buf = bufs[c]
    off = tst[:, [0, 2]].astype(np.float64)              # [P, 2]
    sc = tst[:, [1, 3]].astype(np.float64)
    A2 = (A * sc)[:, :, None].astype(np.float32)
    B2 = (A * off + Bc)[:, :, None].astype(np.float32)
    np.multiply(u8t, A2, out=buf)
    buf += B2
    np.maximum(buf, 0.0, out=buf)
    np.add(buf.transpose(1, 0, 2), xb.reshape(2, P, N),
           out=ob.reshape(2, P, N))


def _run_percore(x, gamma, beta, out):
    """x: [B, C, N] f32. Fills out [B, C, N] f32 completely."""
    import jax
    import threading
    r = _get_runner_percore()
    devs, pool = r["devs"], r["pool"]
    oidx = {name: i for i, name in enumerate(r["out_names"])}
    lock = threading.Lock()
    stats = {}
    box = {}
    ev = threading.Event()

    def work(c):
        x8, xs = _quant_x_core(x[c], c)
        x_c = jax.device_put(x8, devs[c])
        xs_c = jax.device_put(xs, devs[c])
        args = []
        for name in r["in_names"]:
            if name == "x":
                args.append(x_c)
            elif name == "xs":
                args.append(xs_c)
            else:
                args.append(r["wdev"][c][name])
        outs = r["compiled"][c](*args)
        for o in outs:
            try:
                o.copy_to_host_async()
            except Exception:
                pass
        # tstat is tiny and lands well before the 512 KB out tensor: as soon
        # as all 8 are in, global BN coefficients are ready and every core's
        # BN-apply happens inside its own fetch thread (no serial tail).
        tst = np.asarray(outs[oidx["tstat"]])
        with lock:
            stats[c] = tst
            if len(stats) == NCORES:
                s1 = np.zeros((P, 2), np.float64)
                s2 = np.zeros((P, 2), np.float64)
                for k in stats:
                    s1 += stats[k][:, 4:6].astype(np.float64)
                    s2 += stats[k][:, 6:8].astype(np.float64)
                box["A"], box["Bc"] = _bn_coeffs(s1, s2, gamma, beta)
                ev.set()
        u8t = np.asarray(outs[oidx["out"]])
        if not ev.wait(timeout=60.0):
            raise RuntimeError("BN stats barrier timed out")
        _apply_core(u8t, tst, box["A"], box["Bc"], x[c], out[c], c)

    futs = [pool.submit(work, c) for c in range(NCORES)]
    [f.result() for f in futs]


def _run_spmd_fallback(x):
    from concourse.bass_utils import run_bass_kernel_spmd
    nc = _get_nc()
    wh = _host_weights()
    in_maps = []
    for c in range(NCORES):
        x8, xs = _quant_x_core(x[c], c)
        in_maps.append({"x": np.copy(x8), "xs": xs, **wh})
    _CACHE["last_in_maps"] = in_maps
    res = run_bass_kernel_spmd(nc, in_maps, core_ids=list(range(NCORES)))
    _CACHE["last_res"] = res
    return (np.stack([res.results[c]["out"] for c in range(NCORES)]),
            np.stack([res.results[c]["tstat"] for c in range(NCORES)]))


def kernel(x, wq, wv, bv, wt, bt, gamma, beta):
    import hashlib
    x = np.asarray(x, dtype=np.float32)
    wfp = hashlib.md5(
        np.asarray(wq, np.float32).tobytes()
        + np.asarray(wv, np.float32).tobytes()
        + np.asarray(wt, np.float32).tobytes()).hexdigest()
    if _CACHE.get("wfp") != wfp:
        # weights changed (or first call): drop host + device weight caches
        _CACHE.pop("weights", None)
        _CACHE["_raw_w"] = (wq, wv, wt)
        _CACHE["wfp"] = wfp
        rpc = _CACHE.get("runner_pc")
        if rpc is not None:
            import jax
            wh = _host_weights()
            for c in range(NCORES):
                rpc["wdev"][c] = {
                    name: jax.device_put(wh[name], rpc["devs"][c])
                    for name in rpc["in_names"] if name not in ("x", "xs")}

    out = np.empty((B, C, N), np.float32)
    done = False
    if _CACHE.get("_pc_fail_count", 0) < 2:
        try:
            _run_percore(x, gamma, beta, out)
            _CACHE["_pc_fail_count"] = 0
            done = True
        except Exception as e:
            import traceback
            _CACHE["_pc_fail_count"] = _CACHE.get("_pc_fail_count", 0) + 1
            print("percore path failed, falling back:", repr(e)[:300],
                  file=sys.stderr)
            traceback.print_exc()
    if not done:
        u8t, tstat = _run_spmd_fallback(x)
        s1 = np.zeros((P, 2), np.float64)
        s2 = np.zeros((P, 2), np.float64)
        for b in range(B):
            s1 += tstat[b][:, 4:6].astype(np.float64)
            s2 += tstat[b][:, 6:8].astype(np.float64)
        A, Bc = _bn_coeffs(s1, s2, gamma, beta)
        for b in range(B):
            _apply_core(u8t[b], tstat[b], A, Bc, x[b], out[b], b)
    return out
